# revision 1
# baseline (speedup 1.0000x reference)
"""EventRNN (sparse_attention) Trainium2 Bass kernel.

Full-input contract: kernel(**inputs) takes the complete arrays from
setup_inputs() and returns the full (h_new[None], c_new[None]) tuple.

Sharding: data-parallel over batch B=32 across 8 NeuronCores (4 batches
per core); all weights replicated. Host-side prep is layout-only
(transposes / slicing / bool->additive mask); all FLOPs run on device.

Engine/dtype choices: big tensors (features, features_proj, LSTM weights)
ship as bf16 (memory-bound problem; references are bf16-envelope); PSUM
accumulation and softmax stay fp32; small attention logits matmuls use
fp32r. The additive mask is folded into the logits PSUM via a K=1 matmul.
LSTM bias rides the fused gates matmul as a 17th ones-row k-chunk.

Device program per core (b_loc = 4):
  phase A: q = h @ w_h2a.T + b_h2a  and  beta = sigmoid(h @ w_sel.T + b_sel)
           as PE matvecs in [d, b] layout.
  phase B: for each (batch, half):
             H = relu(projT_tile + q)          ACT, per-partition bias
             logits = w_att.T @ H              PE, contract over D in PSUM
             softmax row with additive mask    DVE reduce + ACT exp(accum)
             alphaT via PE transpose
             ctx = alphaT.T @ feats_tiles      PE, contract over L
           fc = beta/sum-scaled (past_ctx + future_ctx)
  phase C: gates = [cap|fc|feat|h] @ [W_ih|W_hh].T + b   one PE matmul chain
           LSTM elementwise on ACT/DVE, DMA h_new/c_new out.
"""

import numpy as np

import concourse.bacc as bacc
import concourse.mybir as mybir
import concourse.tile as tile
import concourse.masks as masks
from concourse.bass_utils import run_bass_kernel_spmd

F32 = mybir.dt.float32
F32R = mybir.dt.float32r
BF16 = mybir.dt.bfloat16
AF = mybir.ActivationFunctionType
ALU = mybir.AluOpType

B, L, D, H = 32, 2048, 512, 512
N_CORES = 8
B_LOC = B // N_CORES          # 4 batches per core
FIDX = 1024                   # static feature_idx from setup_inputs()
HALF = L // 2                 # past/future split == 1024
P = 128
DC = D // P                   # 4 d-chunks
KC = (H + 2 * D + H) // P     # 16 k-chunks for the fused LSTM matmul
KC_G = KC + 1                 # +1 bias chunk (ones-row trick)
G4 = 4 * H                    # 2048 gate columns
LC = HALF // P                # 8 L-chunks of 128 per half
LS = HALF // 512              # 2 N-segments of 512 per half


def build_nc():
    nc = bacc.Bacc("TRN2", target_bir_lowering=False, debug=False,
                   num_devices=N_CORES)

    # ---- DRAM I/O ----
    projT = nc.dram_tensor("projT", [B_LOC, D, L], BF16, kind="ExternalInput").ap()
    feats = nc.dram_tensor("feats", [B_LOC, L, D], BF16, kind="ExternalInput").ap()
    WT = nc.dram_tensor("WT", [KC_G * P, G4], BF16, kind="ExternalInput").ap()
    w_h2aT = nc.dram_tensor("w_h2aT", [H, D], BF16, kind="ExternalInput").ap()
    w_pf = nc.dram_tensor("w_pf", [D, 2], F32R, kind="ExternalInput").ap()
    w_selT = nc.dram_tensor("w_selT", [H, 1], BF16, kind="ExternalInput").ap()
    b_h2a = nc.dram_tensor("b_h2a", [D, 1], F32, kind="ExternalInput").ap()
    b_sel = nc.dram_tensor("b_sel", [1, 1], F32, kind="ExternalInput").ap()
    maskadd = nc.dram_tensor("maskadd", [2 * B_LOC, HALF], BF16,
                             kind="ExternalInput").ap()
    capT = nc.dram_tensor("capT", [H, B_LOC], BF16, kind="ExternalInput").ap()
    featT = nc.dram_tensor("featT", [D, B_LOC], BF16, kind="ExternalInput").ap()
    hT = nc.dram_tensor("hT", [H, B_LOC], BF16, kind="ExternalInput").ap()
    c_last = nc.dram_tensor("c_last", [B_LOC, H], F32, kind="ExternalInput").ap()
    h_out = nc.dram_tensor("h_new", [B_LOC, H], F32, kind="ExternalOutput").ap()
    c_out = nc.dram_tensor("c_new", [B_LOC, H], F32, kind="ExternalOutput").ap()

    with tile.TileContext(nc) as tc:
        with tc.tile_pool(name="const", bufs=1) as const, \
             tc.tile_pool(name="wres", bufs=1) as wres:
            # ---- resident constants ----
            ident = const.tile([P, P], F32)
            masks.make_identity(nc, ident[:])
            ones_bf = const.tile([1, 1], BF16)
            nc.gpsimd.memset(ones_bf[:], 1.0)
            madd_sb = const.tile([1, 2 * B_LOC * HALF], BF16)
            nc.sync.dma_start(madd_sb[:], maskadd.rearrange("r l -> (r l)").unsqueeze(0))

            w_h2aT_sb = const.tile([P, H // P, D], BF16)
            nc.sync.dma_start(w_h2aT_sb[:], w_h2aT.rearrange("(c p) n -> p c n", p=P))
            w_pf_sb = const.tile([P, DC, 2], F32R)
            nc.sync.dma_start(w_pf_sb[:], w_pf.rearrange("(c p) n -> p c n", p=P))
            w_selT_sb = const.tile([P, H // P, 1], BF16)
            nc.sync.dma_start(w_selT_sb[:], w_selT.rearrange("(c p) n -> p c n", p=P))
            b_h2a_sb = const.tile([P, DC], F32)
            nc.sync.dma_start(b_h2a_sb[:], b_h2a.rearrange("(c p) n -> p (c n)", p=P))
            b_sel_sb = const.tile([1, 1], F32)
            nc.sync.dma_start(b_sel_sb[:], b_sel[:])
            # xhT = [caption | fc | feature | h_last] transposed: [128, 16, 4]
            xhT = const.tile([P, KC_G, B_LOC], BF16)
            nc.gpsimd.memset(xhT[:, 16, :], 0.0)
            nc.gpsimd.memset(xhT[0:1, 16, :], 1.0)
            nc.sync.dma_start(xhT[:, 0:4, :], capT.rearrange("(c p) n -> p c n", p=P))
            nc.sync.dma_start(xhT[:, 8:12, :], featT.rearrange("(c p) n -> p c n", p=P))
            nc.sync.dma_start(xhT[:, 12:16, :], hT.rearrange("(c p) n -> p c n", p=P))

            # resident LSTM weights [128, 16, 2048] (128 KB / partition)
            WT_sb = wres.tile([P, KC_G, G4], BF16)

            # softmax / context workspace (all partition-base-0;
            # per-(b,half) scalars live in the FREE dim, r = h*4+b)
            negm = const.tile([1, 2 * B_LOC], F32)
            sums = const.tile([1, 2 * B_LOC], F32)
            recips = const.tile([1, 2 * B_LOC], F32)
            svals = const.tile([1, 2 * B_LOC], F32)
            alphaT = const.tile([P, 2, LC, B_LOC], BF16)
            qb = const.tile([P, DC * B_LOC], F32)
            beta_sb = const.tile([1, B_LOC], F32)

            # ================= phase A: q and beta matvecs =================
            with tc.tile_pool(name="psA", bufs=1, space="PSUM") as psA:
                q_ps = psA.tile([P, DC * B_LOC], F32)
                beta_ps = psA.tile([1, B_LOC], F32)
                for dc in range(DC):
                    for kc in range(H // P):
                        nc.tensor.matmul(
                            q_ps[:, dc * B_LOC:(dc + 1) * B_LOC],
                            w_h2aT_sb[:, kc, dc * P:(dc + 1) * P],
                            xhT[:, 12 + kc, :],
                            start=(kc == 0), stop=(kc == H // P - 1))
                    nc.scalar.activation(
                        qb[:, dc * B_LOC:(dc + 1) * B_LOC],
                        q_ps[:, dc * B_LOC:(dc + 1) * B_LOC],
                        AF.Identity, bias=b_h2a_sb[:, dc:dc + 1])
                for kc in range(H // P):
                    nc.tensor.matmul(beta_ps[:], w_selT_sb[:, kc, :],
                                     xhT[:, 12 + kc, :],
                                     start=(kc == 0), stop=(kc == H // P - 1))
                nc.scalar.activation(beta_sb[:], beta_ps[:], AF.Sigmoid,
                                     bias=b_sel_sb[0:1, 0:1])

            # ================= phase B: attention =================
            with tc.tile_pool(name="proj", bufs=4) as projp, \
                 tc.tile_pool(name="hatt", bufs=6) as hattp, \
                 tc.tile_pool(name="fpool", bufs=4) as fpool, \
                 tc.tile_pool(name="rowp", bufs=3) as rowp, \
                 tc.tile_pool(name="fcpool", bufs=2) as fcpool, \
                 tc.tile_pool(name="pslog", bufs=1, space="PSUM") as pslog, \
                 tc.tile_pool(name="pst", bufs=1, space="PSUM") as pst, \
                 tc.tile_pool(name="psctx", bufs=1, space="PSUM") as psctx, \
                 tc.tile_pool(name="psg", bufs=2, space="PSUM") as psg:

                # fused LSTM gates accumulate during attention; each
                # k-chunk's matmuls are emitted right after its WT DMA
                g_ps1 = psg.tile([B_LOC, 2 * H], F32, tag="g")
                g_ps2 = psg.tile([B_LOC, 2 * H], F32, tag="g")

                fcA = {}
                for b in range(B_LOC):
                    for h in range(2):
                        r = h * B_LOC + b
                        # interleave resident-weight loads with the big loop
                        lg_ps = pslog.tile([1, HALF], F32)
                        # preload additive mask into the logits psum via a
                        # K=1 matmul; logits then accumulate on top
                        for ls in range(LS):
                            nc.tensor.matmul(
                                lg_ps[:, ls * 512:(ls + 1) * 512],
                                ones_bf[0:1, 0:1],
                                madd_sb[0:1, r * HALF + ls * 512:
                                        r * HALF + (ls + 1) * 512],
                                start=True, stop=False)
                        hatts = {}
                        for dp in range(DC // 2):
                            projt = projp.tile([P, 2, HALF], BF16)
                            nc.sync.dma_start(
                                projt[:],
                                projT[b, dp * 2 * P:(dp + 1) * 2 * P,
                                      h * HALF:(h + 1) * HALF]
                                .rearrange("(j p) l -> p j l", p=P))
                            for jj in range(2):
                                dc = dp * 2 + jj
                                hatt = hattp.tile([P, HALF], F32R)
                                nc.scalar.activation(
                                    hatt[:], projt[:, jj, :], AF.Relu,
                                    bias=qb[:, dc * B_LOC + b:
                                            dc * B_LOC + b + 1])
                                hatts[dc] = hatt
                        for ls in range(LS):
                            for dc in range(DC):
                                nc.tensor.matmul(
                                    lg_ps[:, ls * 512:(ls + 1) * 512],
                                    w_pf_sb[:, dc, h:h + 1],
                                    hatts[dc][:, ls * 512:(ls + 1) * 512],
                                    start=False, stop=(dc == DC - 1))
                        # row softmax straight from psum
                        nc.vector.tensor_reduce(
                            negm[0:1, r:r + 1], lg_ps[0:1, :],
                            axis=mybir.AxisListType.X, op=ALU.max, negate=True)
                        alpha_r = rowp.tile([1, HALF], F32, tag="alpha")
                        nc.scalar.activation(
                            alpha_r[:], lg_ps[0:1, :], AF.Exp,
                            bias=negm[0:1, r:r + 1],
                            accum_out=sums[0:1, r:r + 1])
                        nc.vector.reciprocal(recips[0:1, r:r + 1],
                                             sums[0:1, r:r + 1])
                        nc.vector.tensor_tensor(svals[0:1, r:r + 1],
                                                recips[0:1, r:r + 1],
                                                beta_sb[0:1, b:b + 1],
                                                op=ALU.mult)
                        # transpose alpha row into [128, lc] columns
                        for lc in range(LC):
                            tr_ps = pst.tile([P, 1], F32)
                            nc.tensor.transpose(
                                tr_ps[:, 0:1],
                                alpha_r[0:1, lc * P:(lc + 1) * P],
                                ident[0:1, 0:1])
                            nc.vector.tensor_copy(alphaT[:, h, lc, b:b + 1],
                                                  tr_ps[:])
                        # context matvec, contract over L
                        ctx_ps = psctx.tile([1, D], F32)
                        for lq in range(2):
                            featst = fpool.tile([P, 4, D], BF16)
                            nc.sync.dma_start(
                                featst[:],
                                feats[b, h * HALF + lq * 4 * P:
                                      h * HALF + (lq + 1) * 4 * P, :]
                                .rearrange("(j p) d -> p j d", p=P))
                            for jj in range(4):
                                lc = lq * 4 + jj
                                nc.tensor.matmul(
                                    ctx_ps[:], alphaT[:, h, lc, b:b + 1],
                                    featst[:, jj, :],
                                    start=(lc == 0), stop=(lc == LC - 1))
                        if h == 0:
                            # stash s_p * ctx_p, freeing the psum tile
                            fcA_b = fcpool.tile([1, D], F32, tag="fcA", bufs=4)
                            nc.vector.tensor_scalar_mul(
                                fcA_b[:], ctx_ps[0:1, :], svals[0:1, b:b + 1])
                            fcA[b] = fcA_b
                        else:
                            # fc_b = s_f * ctx_f + fcA_b, then -> xhT (transposed)
                            fc_b = fcpool.tile([1, D], F32, tag="fcB", bufs=2)
                            nc.vector.scalar_tensor_tensor(
                                fc_b[:], ctx_ps[0:1, :],
                                svals[0:1, B_LOC + b:B_LOC + b + 1], fcA[b][:],
                                op0=ALU.mult, op1=ALU.add)
                            for dc in range(DC):
                                tr_ps = pst.tile([P, 1], F32)
                                nc.tensor.transpose(
                                    tr_ps[:, 0:1],
                                    fc_b[0:1, dc * P:(dc + 1) * P],
                                    ident[0:1, 0:1])
                                nc.vector.tensor_copy(xhT[:, 4 + dc, b:b + 1],
                                                      tr_ps[:])
                        # weight loads + filler gates matmuls at low
                        # priority (end of each iteration body)
                        base = (b * 2 + h) * 2
                        nc.sync.dma_start(
                            WT_sb[:, base:base + 2, :],
                            WT[base * P:(base + 2) * P, :]
                            .rearrange("(j p) n -> p j n", p=P))
                        ws = [base, base + 1] + ([16] if base == 0 else [])
                        if base == 0:
                            nc.sync.dma_start(WT_sb[:, 16, :],
                                              WT[16 * P:17 * P, :])
                        for wkc in ws:
                            if wkc not in (4, 5, 6, 7):
                                for ns in range(2):
                                    nc.tensor.matmul(
                                        g_ps1[:, ns * 512:(ns + 1) * 512],
                                        xhT[:, wkc, :],
                                        WT_sb[:, wkc, ns * 512:(ns + 1) * 512],
                                        start=(wkc == 0), stop=False)
                                    nc.tensor.matmul(
                                        g_ps2[:, ns * 512:(ns + 1) * 512],
                                        xhT[:, wkc, :],
                                        WT_sb[:, wkc,
                                              (2 + ns) * 512:(3 + ns) * 512],
                                        start=(wkc == 0), stop=False)

            # ================= phase C: fc-dependent gates + LSTM ==========
                lstm = const  # reuse the const pool scope for LSTM tiles
                c_last_sb = lstm.tile([B_LOC, H], F32)
                nc.sync.dma_start(c_last_sb[:], c_last[:])

                for ki, kc in enumerate((4, 5, 6, 7)):
                    for ns in range(2):
                        nc.tensor.matmul(
                            g_ps1[:, ns * 512:(ns + 1) * 512],
                            xhT[:, kc, :],
                            WT_sb[:, kc, ns * 512:(ns + 1) * 512],
                            start=False, stop=(ki == 3))
                for ki, kc in enumerate((4, 5, 6, 7)):
                    for ns in range(2):
                        nc.tensor.matmul(
                            g_ps2[:, ns * 512:(ns + 1) * 512],
                            xhT[:, kc, :],
                            WT_sb[:, kc, (2 + ns) * 512:(3 + ns) * 512],
                            start=False, stop=(ki == 3))
                # gate rows reordered [i, f, o, g]; bias folded into matmul
                g_sb = lstm.tile([B_LOC, G4], F32)
                nc.scalar.activation(g_sb[:, 0:2 * H], g_ps1[:, 0:2 * H],
                                     AF.Sigmoid)
                # f * c_last can run while the second gates half accumulates
                c_new = lstm.tile([B_LOC, H], F32)
                nc.vector.tensor_tensor(c_new[:], g_sb[:, H:2 * H], c_last_sb[:],
                                        op=ALU.mult)

                # tanh(x) = 2*sigmoid(2x) - 1: stays on the sigmoid ACT
                # table (avoids two table loads in the latency-critical tail)
                nc.scalar.activation(g_sb[:, 3 * H:4 * H], g_ps2[:, H:2 * H],
                                     AF.Sigmoid, scale=2.0)
                nc.vector.tensor_scalar(g_sb[:, 3 * H:4 * H],
                                        g_sb[:, 3 * H:4 * H], 2.0, -1.0,
                                        op0=ALU.mult, op1=ALU.add)
                nc.scalar.activation(g_sb[:, 2 * H:3 * H], g_ps2[:, 0:H],
                                     AF.Sigmoid)

                t2 = lstm.tile([B_LOC, H], F32)
                h_new = lstm.tile([B_LOC, H], F32)
                nc.vector.tensor_tensor(t2[:], g_sb[:, 0:H], g_sb[:, 3 * H:4 * H],
                                        op=ALU.mult)
                nc.vector.tensor_tensor(c_new[:], c_new[:], t2[:], op=ALU.add)
                nc.scalar.activation(t2[:], c_new[:], AF.Sigmoid, scale=2.0)
                nc.vector.tensor_scalar(t2[:], t2[:], 2.0, -1.0,
                                        op0=ALU.mult, op1=ALU.add)
                nc.vector.tensor_tensor(h_new[:], g_sb[:, H * 2:H * 3], t2[:],
                                        op=ALU.mult)

                nc.sync.dma_start(c_out[:], c_new[:])
                nc.sync.dma_start(h_out[:], h_new[:])

    nc.compile()
    return nc


_NC_CACHE = None


def _get_nc():
    global _NC_CACHE
    if _NC_CACHE is None:
        _NC_CACHE = build_nc()
    return _NC_CACHE


def make_in_maps(features, features_proj, hidden_states, cell_states,
                 caption_hidden_states, w_h2a, b_h2a, w_patt, b_patt,
                 w_fatt, b_fatt, w_sel, b_sel, w_ih, w_hh, b_ih, b_hh,
                 mask, feature_idx):
    assert int(feature_idx) == FIDX
    import ml_dtypes
    f32 = np.float32
    bf16 = ml_dtypes.bfloat16
    features = np.asarray(features, f32)
    features_proj = np.asarray(features_proj, f32)
    h_last = np.asarray(hidden_states, f32)[-1]          # [B, H]
    c_last = np.asarray(cell_states, f32)[-1]            # [B, H]
    cap = np.asarray(caption_hidden_states, f32)         # [B, H]
    mask = np.asarray(mask)

    # shared (replicated) tensors — layout-only host prep
    Wfull = np.concatenate([np.asarray(w_ih, f32), np.asarray(w_hh, f32)], axis=1)
    gate_perm = np.r_[0:512, 512:1024, 1536:2048, 1024:1536]
    b_ihh = (np.asarray(b_ih, f32) + np.asarray(b_hh, f32))[gate_perm]
    WTf = np.zeros((KC_G * 128, 4 * H), f32)
    WTf[0:2048] = Wfull[gate_perm].T
    WTf[2048] = b_ihh
    WT = np.ascontiguousarray(WTf).astype(bf16)
    w_h2aT = np.ascontiguousarray(np.asarray(w_h2a, f32).T).astype(bf16)
    w_pf = np.ascontiguousarray(
        np.stack([np.asarray(w_patt, f32)[0], np.asarray(w_fatt, f32)[0]], axis=1))
    w_selT = np.ascontiguousarray(np.asarray(w_sel, f32).T).astype(bf16)
    b_h2a_c = np.ascontiguousarray(np.asarray(b_h2a, f32)[:, None])  # [D, 1]
    b_sel_c = np.asarray(b_sel, f32).reshape(1, 1)
    # additive mask, rows (half, b): 0 where visible, -1e30 where masked
    madd = np.where(mask, f32(0), f32(-1e30)).astype(bf16)           # [B, L]

    in_maps = []
    for c in range(N_CORES):
        sl = slice(c * B_LOC, (c + 1) * B_LOC)
        m = madd[sl].reshape(B_LOC, 2, HALF).transpose(1, 0, 2)      # [2, 4, HALF]
        in_maps.append({
            "projT": np.ascontiguousarray(features_proj[sl].transpose(0, 2, 1)).astype(bf16),
            "feats": np.ascontiguousarray(features[sl]).astype(bf16),
            "WT": WT,
            "w_h2aT": w_h2aT,
            "w_pf": w_pf,
            "w_selT": w_selT,
            "b_h2a": b_h2a_c,
            "b_sel": b_sel_c,
            "maskadd": np.ascontiguousarray(m.reshape(2 * B_LOC, HALF)),
            "capT": np.ascontiguousarray(cap[sl].T).astype(bf16),
            "featT": np.ascontiguousarray(features[sl, FIDX, :].T).astype(bf16),
            "hT": np.ascontiguousarray(h_last[sl].T).astype(bf16),
            "c_last": np.ascontiguousarray(c_last[sl]),
        })
    return in_maps


def run(trace=False, **inputs):
    nc = _get_nc()
    in_maps = make_in_maps(**inputs)
    res = run_bass_kernel_spmd(nc, in_maps, core_ids=list(range(N_CORES)),
                               trace=trace)
    h = np.concatenate([res.results[c]["h_new"] for c in range(N_CORES)], axis=0)
    c = np.concatenate([res.results[c]["c_new"] for c in range(N_CORES)], axis=0)
    return (h[None], c[None]), res


def kernel(**inputs):
    out, _ = run(trace=False, **inputs)
    return out



# revision 8
# speedup vs baseline: 1.8036x; 1.8036x over previous
"""EventRNN (sparse_attention) Trainium2 Bass kernel — v3.

Full-input contract: kernel(**inputs) takes the complete arrays from
setup_inputs() and returns the full (h_new[None], c_new[None]) tuple.

Sharding: data-parallel over batch B=32 across 8 NeuronCores (4 batches
per core); all weights replicated. Host-side prep is layout-only
(transposes / slicing / dtype casts / bool->additive mask).

Design (DMA-roofline focused):
 - features / features_proj / w_h2a / w_sel / h(for matvecs) ship as
   fp8e4m3 (validated on host: final rel err ~2.4e-3 vs 2.7e-3 for the
   all-bf16 baseline). LSTM W stays bf16 (fp8 W alone costs 3.5e-2).
   Per-core DMA drops 25.8 MB -> 17.1 MB.
 - DMA order: packed consts (4 DMAs) -> all proj/feats (the attention
   pipeline's food) -> LSTM W chunk pairs, fc-fed chunks last. The 2KB
   psum-bank accumulation runs under single start=True preloads (bias /
   mask matmuls) so 272 gate matmuls + 256 logits matmuls can land in
   shared banks.
 - logits are computed TRANSPOSED: lhsT = 128x128 relu'd-proj chunks
   (stationary), rhs = attention weight column -> logits in PSUM as
   [128(l), 8(lc)]. Softmax exp runs on ACT at free-size 8 (128x fewer
   cycles than a [1,1024] row), alpha needs no PE transposes, no
   max-subtract (bounded logits, fp32 psum), alpha written directly as
   fp8 for the ctx matvec.
 - gates matmul runs output-transposed: lhsT = W.T 128x128 chunks
   (stationary), rhs = xhT [128,4]; psum [128(g), 64]. LSTM elementwise
   tail in the transposed layout; host un-transposes [128, 32] output.
 - one ACT table for the whole kernel (exp_and_others: relu/exp/
   identity/tanh); sigmoids via 0.5*tanh(x/2)+0.5. No table switches.
"""

import numpy as np

import concourse.bacc as bacc
import concourse.mybir as mybir
import concourse.tile as tile
import concourse.masks as masks
from concourse.bass_utils import run_bass_kernel_spmd

F32 = mybir.dt.float32
BF16 = mybir.dt.bfloat16
F8 = mybir.dt.float8e4
AF = mybir.ActivationFunctionType
ALU = mybir.AluOpType

B, L, D, H = 32, 2048, 512, 512
N_CORES = 8
B_LOC = B // N_CORES          # 4 batches per core
FIDX = 1024                   # static feature_idx from setup_inputs()
HALF = L // 2                 # past/future split == 1024
P = 128
DC = D // P                   # 4 d-chunks
HC = H // P                   # 4 h-chunks
LC = HALF // P                # 8 l-chunks per half
KC = 16                       # k-chunks of the fused LSTM matmul (2048/128)
GC = 16                       # gate-column chunks (4H/128)
G4 = 4 * H
S_W = 1024.0                  # fp8 scale for the small matvec weights
# xhT k-chunk order: [cap 0:4 | feature 4:8 | h 8:12 | fc 12:16] so the
# attention-context-dependent chunks come last (their W pair is also the
# last DMA; the gates tail then runs right as the final W bytes land).
FC_KCS = (12, 13, 14, 15)
WT_PAIRS = [(0, 1), (2, 3), (4, 5), (6, 7), (8, 9), (10, 11), (12, 13),
            (14, 15)]
# packed-const offsets
F8C_WSEL = 2048               # f8 pack: w_h2a [0:2048], w_sel, hT8
F8C_HT = 2052
BFC_WPF = 64                  # bf16 pack: xhT [0:64], w_pf [64:72]
SC_BRHS = 128                 # small pack rows=16: biasT2 [0:128], brhs,
SC_ID8 = 192                  # ident8 [192:200], madd2 [200:1224] (rows 0:8)
SC_MADD = 200


def build_nc():
    nc = bacc.Bacc("TRN2", target_bir_lowering=False, debug=False,
                   num_devices=N_CORES)

    # ---- DRAM I/O ----
    projT = nc.dram_tensor("projT", [B_LOC, D, L], F8, kind="ExternalInput").ap()
    feats = nc.dram_tensor("feats", [B_LOC, L, D], F8, kind="ExternalInput").ap()
    WT = nc.dram_tensor("WT", [12 * P, G4], BF16, kind="ExternalInput").ap()
    WT8 = nc.dram_tensor("WT8", [4 * P, G4], F8, kind="ExternalInput").ap()
    f8c = nc.dram_tensor("f8c", [P, 2068], F8, kind="ExternalInput").ap()
    f32c = nc.dram_tensor("f32c", [P, 21], F32, kind="ExternalInput").ap()
    bfc = nc.dram_tensor("bfc", [P, 72], BF16, kind="ExternalInput").ap()
    smallc = nc.dram_tensor("smallc", [16, 1224], BF16, kind="ExternalInput").ap()
    out_d = nc.dram_tensor("out", [P, 32], F32, kind="ExternalOutput").ap()

    with tile.TileContext(nc) as tc:
        with tc.tile_pool(name="const", bufs=1) as const, \
             tc.tile_pool(name="wres", bufs=1) as wres:
            # ---- resident constants / packed small inputs ----
            ident = const.tile([P, P], F32)
            masks.make_identity(nc, ident[:])
            ones_f8 = const.tile([P, 1], F8)
            nc.gpsimd.memset(ones_f8[:], 1.0)

            f8c_sb = const.tile([P, 2068], F8)
            f32c_sb = const.tile([P, 21], F32)
            smallc_sb = const.tile([16, 1224], BF16)
            # xhT lives inside the bf16 pack; fc x-chunks go to the fp8 tile
            bfc_sb = const.tile([P, 72], BF16)
            xf8_sb = const.tile([P, 16], F8)

            # resident LSTM weights: 12 bf16 chunks + 4 fp8 fc chunks
            WT_sb = wres.tile([P, 12, G4], BF16)
            WT8_sb = wres.tile([P, 4, G4], F8)

            # scalars along free dims, r = b*2 + h
            qb = const.tile([P, DC, B_LOC], F32)
            tb = const.tile([1, B_LOC], F32)
            beta_sb = const.tile([1, B_LOC], F32)
            sums_sb = const.tile([1, 2 * B_LOC], F32)
            recips = const.tile([1, 2 * B_LOC], F32)
            svals = const.tile([1, 2 * B_LOC], F32)

            with tc.tile_pool(name="proj", bufs=8) as projp, \
                 tc.tile_pool(name="hatt", bufs=2) as hattp, \
                 tc.tile_pool(name="fpool", bufs=8) as fpool, \
                 tc.tile_pool(name="alphap", bufs=3) as alphap, \
                 tc.tile_pool(name="fcp", bufs=2) as fcp:

                # ---- DMA order: proj0, packed consts, then the stream;
                # bf16 W pairs next-to-last, the fp8 fc W chunk dead last ----
                projts, featsts = [], []
                for u in range(2 * B_LOC):
                    b, h = divmod(u, 2)
                    projt = projp.tile([P, DC, HALF], F8)
                    nc.sync.dma_start(
                        projt[:],
                        projT[b, :, h * HALF:(h + 1) * HALF]
                        .rearrange("(c p) l -> p c l", p=P))
                    projts.append(projt)
                    if u == 0:
                        nc.sync.dma_start(f8c_sb[:], f8c[:])
                        nc.sync.dma_start(f32c_sb[:], f32c[:])
                        nc.sync.dma_start(smallc_sb[:], smallc[:])
                    featst = fpool.tile([P, LC, D], F8)
                    nc.sync.dma_start(
                        featst[:],
                        feats[b, h * HALF:(h + 1) * HALF, :]
                        .rearrange("(c p) d -> p c d", p=P))
                    featsts.append(featst)
                    if u == 0:
                        nc.sync.dma_start(bfc_sb[:], bfc[:])
                for k0, k1 in WT_PAIRS[:6]:
                    nc.sync.dma_start(
                        WT_sb[:, k0:k1 + 1, :],
                        WT[k0 * P:(k1 + 1) * P, :]
                        .rearrange("(j p) n -> p j n", p=P))
                nc.sync.dma_start(
                    WT8_sb[:],
                    WT8.rearrange("(j p) n -> p j n", p=P))

                # ============ phase A: q and beta matvecs (fp8, x1024) ======
                with tc.tile_pool(name="psA", bufs=2, space="PSUM") as psA:
                    for dc in range(DC):
                        qt = psA.tile([P, B_LOC], F32, tag="q")
                        for hc in range(HC):
                            nc.tensor.matmul(
                                qt[:],
                                f8c_sb[:, hc * D + dc * P:
                                       hc * D + (dc + 1) * P],
                                f8c_sb[:, F8C_HT + hc * B_LOC:
                                       F8C_HT + (hc + 1) * B_LOC],
                                start=(hc == 0), stop=(hc == HC - 1))
                        nc.scalar.activation(qb[:, dc, :], qt[:], AF.Identity,
                                             bias=f32c_sb[:, 16 + dc:17 + dc],
                                             scale=1.0 / S_W)
                    bps = psA.tile([1, B_LOC], F32, tag="beta", bufs=1)
                    for hc in range(HC):
                        nc.tensor.matmul(
                            bps[:],
                            f8c_sb[:, F8C_WSEL + hc:F8C_WSEL + hc + 1],
                            f8c_sb[:, F8C_HT + hc * B_LOC:
                                   F8C_HT + (hc + 1) * B_LOC],
                            start=(hc == 0), stop=(hc == HC - 1))
                    # beta = sigmoid(z) = 0.5*tanh(z/2) + 0.5 (one ACT table)
                    nc.scalar.activation(tb[:], bps[:], AF.Tanh,
                                         bias=f32c_sb[0:1, 20:21],
                                         scale=0.5 / S_W)
                    # beta' = 64*sigmoid(z): the 64 is the fp8 fc x-scale
                    nc.vector.tensor_scalar(beta_sb[:], tb[:], 32.0, 32.0,
                                            op0=ALU.mult, op1=ALU.add)

                # ================= phase B: attention + gates + LSTM ========
                with tc.tile_pool(name="pslog", bufs=2, space="PSUM") as pslog, \
                     tc.tile_pool(name="pssum", bufs=1, space="PSUM") as pssum, \
                     tc.tile_pool(name="psctx", bufs=2, space="PSUM") as psctx, \
                     tc.tile_pool(name="psg", bufs=1, space="PSUM") as psg:

                    # gates psum [128(g), gc*4+b]; single start=True via the
                    # K=16 bias-broadcast preload matmul
                    g_ps = psg.tile([P, GC * B_LOC], F32, tag="g")
                    nc.tensor.matmul(g_ps[:], smallc_sb[:, 0:P],
                                     smallc_sb[:, SC_BRHS:SC_BRHS + GC * B_LOC],
                                     start=True, stop=False)

                    fcA = {}
                    for b in range(B_LOC):
                        for h in range(2):
                            u = b * 2 + h
                            r = u
                            projt, featst = projts[u], featsts[u]

                            # relu(proj + q) on ACT, bf16 out
                            hatt = hattp.tile([P, DC, HALF], BF16)
                            for dc in range(DC):
                                nc.scalar.activation(
                                    hatt[:, dc, :], projt[:, dc, :], AF.Relu,
                                    bias=qb[:, dc, b:b + 1])

                            # mask preload -> logitsT psum [128, 8], one start
                            lg_ps = pslog.tile([P, LC], F32)
                            nc.tensor.matmul(
                                lg_ps[:],
                                smallc_sb[0:LC, SC_MADD + r * P:
                                          SC_MADD + (r + 1) * P],
                                smallc_sb[0:LC, SC_ID8:SC_ID8 + LC],
                                start=True, stop=False)
                            # logitsT: stationary hatt chunks, moving w column
                            for dc in range(DC):
                                for lc in range(LC):
                                    nc.tensor.matmul(
                                        lg_ps[:, lc:lc + 1],
                                        hatt[:, dc, lc * P:(lc + 1) * P],
                                        bfc_sb[:, BFC_WPF + dc * 2 + h:
                                               BFC_WPF + dc * 2 + h + 1],
                                        start=False,
                                        stop=(dc == DC - 1 and lc == LC - 1))

                            # softmax: exp on [128, 8]; alpha straight to fp8
                            alpha_t = alphap.tile([P, LC, 1], F8)
                            nc.scalar.activation(alpha_t[:, :, 0], lg_ps[:],
                                                 AF.Exp)
                            sums_ps = pssum.tile([1, LC], F32, tag="sums")
                            nc.tensor.matmul(sums_ps[:], ones_f8[:],
                                             alpha_t[:, :, 0],
                                             start=True, stop=True)
                            nc.vector.tensor_reduce(
                                sums_sb[0:1, r:r + 1], sums_ps[0:1, :],
                                axis=mybir.AxisListType.X, op=ALU.add)
                            nc.vector.reciprocal(recips[0:1, r:r + 1],
                                                 sums_sb[0:1, r:r + 1])
                            nc.vector.tensor_tensor(svals[0:1, r:r + 1],
                                                    recips[0:1, r:r + 1],
                                                    beta_sb[0:1, b:b + 1],
                                                    op=ALU.mult)

                            # ctx[1, 512] += alpha_lc.T @ feats_lc (fp8)
                            ctx_ps = psctx.tile([1, D], F32)
                            for lc in range(LC):
                                nc.tensor.matmul(
                                    ctx_ps[:], alpha_t[:, lc, :],
                                    featst[:, lc, :],
                                    start=(lc == 0), stop=(lc == LC - 1))
                            if h == 0:
                                fcA_b = fcp.tile([1, D], F32, tag="fcA", bufs=2)
                                nc.vector.tensor_scalar_mul(
                                    fcA_b[:], ctx_ps[0:1, :],
                                    svals[0:1, r:r + 1])
                                fcA[b] = fcA_b
                            else:
                                fc_b = fcp.tile([1, D], F32, tag="fcB", bufs=2)
                                nc.vector.scalar_tensor_tensor(
                                    fc_b[:], ctx_ps[0:1, :], svals[0:1, r:r + 1],
                                    fcA[b][:], op0=ALU.mult, op1=ALU.add)
                                for dc in range(DC):
                                    tr_ps = pssum.tile([P, 1], F32, tag="tr",
                                                       bufs=2)
                                    nc.tensor.transpose(
                                        tr_ps[:, 0:1],
                                        fc_b[0:1, dc * P:(dc + 1) * P],
                                        ident[0:1, 0:1])
                                    nc.vector.tensor_copy(
                                        xf8_sb[:, dc * B_LOC + b:
                                               dc * B_LOC + b + 1],
                                        tr_ps[:])

                    # ========== phase C: gates + LSTM tail ==================
                    # gates matmuls in WT-arrival order; fp8 fc chunks (and
                    # the psum stop) land last, as the final W bytes arrive
                    for kc in range(12):
                        for gc in range(GC):
                            nc.tensor.matmul(
                                g_ps[:, gc * B_LOC:(gc + 1) * B_LOC],
                                WT_sb[:, kc, gc * P:(gc + 1) * P],
                                bfc_sb[:, kc * B_LOC:(kc + 1) * B_LOC],
                                start=False, stop=False)
                    for dc in range(4):
                        for gc in range(GC):
                            nc.tensor.matmul(
                                g_ps[:, gc * B_LOC:(gc + 1) * B_LOC],
                                WT8_sb[:, dc, gc * P:(gc + 1) * P],
                                xf8_sb[:, dc * B_LOC:(dc + 1) * B_LOC],
                                start=False,
                                stop=(dc == 3 and gc == GC - 1))

                    # transposed LSTM tail; gate rows (permuted) = [i,f,o,g]
                    lstm = const
                    t_if = lstm.tile([P, 32], F32)
                    t_o = lstm.tile([P, 16], F32)
                    t_g = lstm.tile([P, 16], F32)
                    t2 = lstm.tile([P, 16], F32)
                    c2 = lstm.tile([P, 16], F32)
                    outt = lstm.tile([P, 32], F32)   # [c_new | h_new]

                    # sigmoid(x) = 0.5*tanh(x/2)+0.5 for i, f, o; tanh
                    # for g; psum carries S_G=256-scaled gates
                    nc.scalar.activation(t_if[:], g_ps[:, 0:32], AF.Tanh,
                                         scale=0.5 / 256.0)
                    nc.scalar.activation(t_g[:], g_ps[:, 48:64], AF.Tanh,
                                         scale=1.0 / 256.0)
                    nc.scalar.activation(t_o[:], g_ps[:, 32:48], AF.Tanh,
                                         scale=0.5 / 256.0)
                    nc.vector.tensor_scalar(t_if[:], t_if[:], 0.5, 0.5,
                                            op0=ALU.mult, op1=ALU.add)
                    # c_new = sig_f * c_last + sig_i * tanh_g
                    nc.vector.tensor_tensor(outt[:, 0:16], t_if[:, 16:32],
                                            f32c_sb[:, 0:16], op=ALU.mult)
                    nc.vector.tensor_tensor(c2[:], t_if[:, 0:16], t_g[:],
                                            op=ALU.mult)
                    nc.vector.tensor_tensor(outt[:, 0:16], outt[:, 0:16],
                                            c2[:], op=ALU.add)
                    nc.vector.tensor_scalar(t_o[:], t_o[:], 0.5, 0.5,
                                            op0=ALU.mult, op1=ALU.add)
                    nc.scalar.activation(t2[:], outt[:, 0:16], AF.Tanh)
                    nc.vector.tensor_tensor(outt[:, 16:32], t_o[:], t2[:],
                                            op=ALU.mult)
                    nc.sync.dma_start(out_d[:], outt[:])

    nc.compile()
    return nc


_NC_CACHE = None


def _get_nc():
    global _NC_CACHE
    if _NC_CACHE is None:
        _NC_CACHE = build_nc()
    return _NC_CACHE


def split_out(arr):
    """[128, 32] device layout -> (h_new [B_LOC, H], c_new [B_LOC, H])."""
    a = np.asarray(arr, np.float32).reshape(P, 2, HC, B_LOC)
    c = np.ascontiguousarray(a[:, 0].transpose(2, 1, 0).reshape(B_LOC, H))
    h = np.ascontiguousarray(a[:, 1].transpose(2, 1, 0).reshape(B_LOC, H))
    return h, c


def make_in_maps(features, features_proj, hidden_states, cell_states,
                 caption_hidden_states, w_h2a, b_h2a, w_patt, b_patt,
                 w_fatt, b_fatt, w_sel, b_sel, w_ih, w_hh, b_ih, b_hh,
                 mask, feature_idx):
    assert int(feature_idx) == FIDX
    import ml_dtypes
    f32 = np.float32
    bf16 = ml_dtypes.bfloat16
    f8 = ml_dtypes.float8_e4m3
    features = np.asarray(features, f32)
    features_proj = np.asarray(features_proj, f32)
    h_last = np.asarray(hidden_states, f32)[-1]          # [B, H]
    c_last = np.asarray(cell_states, f32)[-1]            # [B, H]
    cap = np.asarray(caption_hidden_states, f32)         # [B, H]
    mask = np.asarray(mask)

    # shared (replicated) tensors — layout-only host prep + dtype casts
    Wfull = np.concatenate([np.asarray(w_ih, f32), np.asarray(w_hh, f32)], axis=1)
    gate_perm = np.r_[0:512, 512:1024, 1536:2048, 1024:1536]   # [i, f, o, g]
    k_perm = np.r_[0:512, 1024:1536, 1536:2048, 512:1024]      # [cap,feat,h,fc]
    b_ihh = (np.asarray(b_ih, f32) + np.asarray(b_hh, f32))[gate_perm] * 256.0
    WTf = Wfull[gate_perm][:, k_perm].T                       # [k, g]
    WTh = np.ascontiguousarray(WTf[0:12 * P] * 256.0).astype(bf16)
    WT8h = np.ascontiguousarray(WTf[12 * P:] * 4.0).astype(f8)

    # small bf16 pack rows=16: biasT2 | brhs | ident8 | madd2 (per core)
    sc_base = np.zeros((16, 1224), f32)
    sc_base[:, 0:P] = b_ihh.reshape(GC, P)
    sc_base[:, SC_BRHS:SC_BRHS + GC * B_LOC] = np.kron(
        np.eye(GC, dtype=f32), np.ones((1, B_LOC), f32))
    sc_base[0:LC, SC_ID8:SC_ID8 + LC] = np.eye(LC, dtype=f32)

    # f8 pack: w_h2a (hc-major) | w_sel | hT8 (hT8 filled per core)
    f8c_w = np.zeros((P, 2068), f32)
    w_h2aT = np.asarray(w_h2a, f32).T * S_W                    # [H, D]
    f8c_w[:, 0:2048] = w_h2aT.reshape(HC, P, D).transpose(1, 0, 2) \
        .reshape(P, 2048)
    f8c_w[:, F8C_WSEL:F8C_WSEL + HC] = (
        np.asarray(w_sel, f32).T * S_W).reshape(HC, P).T

    w_pf = np.stack([np.asarray(w_patt, f32)[0], np.asarray(w_fatt, f32)[0]],
                    axis=1)                                    # [D, 2]
    madd = np.where(mask, f32(0), f32(-1e30))                  # [B, L]

    in_maps = []
    for c in range(N_CORES):
        sl = slice(c * B_LOC, (c + 1) * B_LOC)
        sc = sc_base.copy()
        # madd2[lc, (b*2+h)*128 + p] = madd_loc[b, h*HALF + lc*128 + p]
        m = madd[sl].reshape(B_LOC, 2, LC, P)
        sc[0:LC, SC_MADD:SC_MADD + 2 * B_LOC * P] = (
            m.transpose(2, 0, 1, 3).reshape(LC, 2 * B_LOC * P))

        f8cc = f8c_w.copy()
        f8cc[:, F8C_HT:F8C_HT + 16] = (
            h_last[sl].T.reshape(HC, P, B_LOC).transpose(1, 0, 2)
            .reshape(P, 16))

        bfc = np.zeros((P, 72), f32)
        xh = np.stack([cap[sl], features[sl, FIDX, :], h_last[sl]], axis=0)
        # xh [3, B_LOC, H] -> bfc[p, kc*4+b] for kc blocks cap/feat/h
        bfc[:, 0:48] = xh.reshape(3, B_LOC, HC, P).transpose(3, 0, 2, 1) \
            .reshape(P, 48)
        bfc[:, BFC_WPF:BFC_WPF + 8] = w_pf.reshape(DC, P, 2) \
            .transpose(1, 0, 2).reshape(P, 8)

        f32cc = np.zeros((P, 21), f32)
        f32cc[:, 0:16] = c_last[sl].T.reshape(HC, P, B_LOC) \
            .transpose(1, 0, 2).reshape(P, 16)
        f32cc[:, 16:20] = np.asarray(b_h2a, f32).reshape(DC, P).T
        f32cc[0, 20] = np.asarray(b_sel, f32).reshape(-1)[0] * 0.5

        in_maps.append({
            "projT": np.ascontiguousarray(
                features_proj[sl].transpose(0, 2, 1)).astype(f8),
            "feats": np.ascontiguousarray(features[sl]).astype(f8),
            "WT": WTh,
            "WT8": WT8h,
            "f8c": f8cc.astype(f8),
            "f32c": f32cc,
            "bfc": bfc.astype(bf16),
            "smallc": sc.astype(bf16),
        })
    return in_maps


def run(trace=False, **inputs):
    nc = _get_nc()
    in_maps = make_in_maps(**inputs)
    res = run_bass_kernel_spmd(nc, in_maps, core_ids=list(range(N_CORES)),
                               trace=trace)
    hs, cs = [], []
    for c in range(N_CORES):
        h, cc = split_out(res.results[c]["out"])
        hs.append(h)
        cs.append(cc)
    return (np.concatenate(hs)[None], np.concatenate(cs)[None]), res


def kernel(**inputs):
    out, _ = run(trace=False, **inputs)
    return out


# revision 9
# speedup vs baseline: 1.8690x; 1.0363x over previous
"""EventRNN (sparse_attention) Trainium2 Bass kernel — v3.

Full-input contract: kernel(**inputs) takes the complete arrays from
setup_inputs() and returns the full (h_new[None], c_new[None]) tuple.

Sharding: data-parallel over batch B=32 across 8 NeuronCores (4 batches
per core); all weights replicated. Host-side prep is layout-only
(transposes / slicing / dtype casts / bool->additive mask).

Design (DMA-roofline focused):
 - features / features_proj / w_h2a / w_sel / h(for matvecs) ship as
   fp8e4m3 (validated on host: final rel err ~2.4e-3 vs 2.7e-3 for the
   all-bf16 baseline). LSTM W stays bf16 (fp8 W alone costs 3.5e-2).
   Per-core DMA drops 25.8 MB -> 17.1 MB.
 - DMA order: packed consts (4 DMAs) -> all proj/feats (the attention
   pipeline's food) -> LSTM W chunk pairs, fc-fed chunks last. The 2KB
   psum-bank accumulation runs under single start=True preloads (bias /
   mask matmuls) so 272 gate matmuls + 256 logits matmuls can land in
   shared banks.
 - logits are computed TRANSPOSED: lhsT = 128x128 relu'd-proj chunks
   (stationary), rhs = attention weight column -> logits in PSUM as
   [128(l), 8(lc)]. Softmax exp runs on ACT at free-size 8 (128x fewer
   cycles than a [1,1024] row), alpha needs no PE transposes, no
   max-subtract (bounded logits, fp32 psum), alpha written directly as
   fp8 for the ctx matvec.
 - gates matmul runs output-transposed: lhsT = W.T 128x128 chunks
   (stationary), rhs = xhT [128,4]; psum [128(g), 64]. LSTM elementwise
   tail in the transposed layout; host un-transposes [128, 32] output.
 - one ACT table for the whole kernel (exp_and_others: relu/exp/
   identity/tanh); sigmoids via 0.5*tanh(x/2)+0.5. No table switches.
"""

import numpy as np

import concourse.bacc as bacc
import concourse.mybir as mybir
import concourse.tile as tile
import concourse.masks as masks
from concourse.bass_utils import run_bass_kernel_spmd

F32 = mybir.dt.float32
BF16 = mybir.dt.bfloat16
F8 = mybir.dt.float8e4
F83 = mybir.dt.float8e3
AF = mybir.ActivationFunctionType
ALU = mybir.AluOpType

B, L, D, H = 32, 2048, 512, 512
N_CORES = 8
B_LOC = B // N_CORES          # 4 batches per core
FIDX = 1024                   # static feature_idx from setup_inputs()
HALF = L // 2                 # past/future split == 1024
P = 128
DC = D // P                   # 4 d-chunks
HC = H // P                   # 4 h-chunks
LC = HALF // P                # 8 l-chunks per half
KC = 16                       # k-chunks of the fused LSTM matmul (2048/128)
GC = 16                       # gate-column chunks (4H/128)
G4 = 4 * H
S_W = 1024.0                  # fp8 scale for the small matvec weights
S_G = 128.0                   # global gates scale (W*128, descaled in ACT)
N_E3M4 = 8                    # trailing cap/feat/h W chunks in fp8e3m4
N_BF = 12 - N_E3M4            # leading W chunks kept bf16
# xhT k-chunk order: [cap 0:4 | feature 4:8 | h 8:12 | fc 12:16] so the
# attention-context-dependent chunks come last (their W pair is also the
# last DMA; the gates tail then runs right as the final W bytes land).
FC_KCS = (12, 13, 14, 15)
WT_PAIRS = [(0, 1), (2, 3), (4, 5), (6, 7), (8, 9), (10, 11), (12, 13),
            (14, 15)]
# packed-const offsets
F8C_WSEL = 2048               # f8 pack: w_h2a [0:2048], w_sel, hT8
F8C_HT = 2052
BFC_WPF = 64                  # bf16 pack: xhT [0:64], w_pf [64:72]
SC_BRHS = 128                 # small pack rows=16: biasT2 [0:128], brhs,
SC_ID8 = 192                  # ident8 [192:200], madd2 [200:1224] (rows 0:8)
SC_MADD = 200


def build_nc():
    nc = bacc.Bacc("TRN2", target_bir_lowering=False, debug=False,
                   num_devices=N_CORES)

    # ---- DRAM I/O ----
    projT = nc.dram_tensor("projT", [B_LOC, D, L], F8, kind="ExternalInput").ap()
    feats = nc.dram_tensor("feats", [B_LOC, L, D], F8, kind="ExternalInput").ap()
    WT = nc.dram_tensor("WT", [N_BF * P, G4], BF16, kind="ExternalInput").ap()
    WT83 = nc.dram_tensor("WT83", [N_E3M4 * P, G4], F83,
                          kind="ExternalInput").ap()
    WT8 = nc.dram_tensor("WT8", [4 * P, G4], F8, kind="ExternalInput").ap()
    f8c = nc.dram_tensor("f8c", [P, 2068], F8, kind="ExternalInput").ap()
    f32c = nc.dram_tensor("f32c", [P, 21], F32, kind="ExternalInput").ap()
    bfc = nc.dram_tensor("bfc", [P, 72], BF16, kind="ExternalInput").ap()
    smallc = nc.dram_tensor("smallc", [16, 1224], BF16, kind="ExternalInput").ap()
    out_d = nc.dram_tensor("out", [P, 32], F32, kind="ExternalOutput").ap()

    with tile.TileContext(nc) as tc:
        with tc.tile_pool(name="const", bufs=1) as const, \
             tc.tile_pool(name="wres", bufs=1) as wres:
            # ---- resident constants / packed small inputs ----
            ident = const.tile([P, P], F32)
            masks.make_identity(nc, ident[:])
            ones_f8 = const.tile([P, 1], F8)
            nc.gpsimd.memset(ones_f8[:], 1.0)

            f8c_sb = const.tile([P, 2068], F8)
            f32c_sb = const.tile([P, 21], F32)
            smallc_sb = const.tile([16, 1224], BF16)
            # xhT lives inside the bf16 pack; fc x-chunks go to the fp8 tile
            bfc_sb = const.tile([P, 72], BF16)
            xf8_sb = const.tile([P, 16], F8)

            # resident LSTM weights: bf16 + e3m4 cap/feat/h + e4m3 fc
            WT_sb = wres.tile([P, N_BF, G4], BF16)
            WT83_sb = wres.tile([P, N_E3M4, G4], F83)
            WT8_sb = wres.tile([P, 4, G4], F8)

            # scalars along free dims, r = b*2 + h
            qb = const.tile([P, DC, B_LOC], F32)
            tb = const.tile([1, B_LOC], F32)
            beta_sb = const.tile([1, B_LOC], F32)
            sums_sb = const.tile([1, 2 * B_LOC], F32)
            recips = const.tile([1, 2 * B_LOC], F32)
            svals = const.tile([1, 2 * B_LOC], F32)

            with tc.tile_pool(name="proj", bufs=8) as projp, \
                 tc.tile_pool(name="hatt", bufs=2) as hattp, \
                 tc.tile_pool(name="fpool", bufs=8) as fpool, \
                 tc.tile_pool(name="alphap", bufs=3) as alphap, \
                 tc.tile_pool(name="fcp", bufs=2) as fcp:

                # ---- DMA order: proj0, packed consts, then the stream;
                # bf16 W pairs next-to-last, the fp8 fc W chunk dead last ----
                projts, featsts = [], []
                for u in range(2 * B_LOC):
                    b, h = divmod(u, 2)
                    projt = projp.tile([P, DC, HALF], F8)
                    nc.sync.dma_start(
                        projt[:],
                        projT[b, :, h * HALF:(h + 1) * HALF]
                        .rearrange("(c p) l -> p c l", p=P))
                    projts.append(projt)
                    if u == 0:
                        nc.sync.dma_start(f8c_sb[:], f8c[:])
                        nc.sync.dma_start(f32c_sb[:], f32c[:])
                        nc.sync.dma_start(smallc_sb[:], smallc[:])
                    featst = fpool.tile([P, LC, D], F8)
                    nc.sync.dma_start(
                        featst[:],
                        feats[b, h * HALF:(h + 1) * HALF, :]
                        .rearrange("(c p) d -> p c d", p=P))
                    featsts.append(featst)
                    if u == 0:
                        nc.sync.dma_start(bfc_sb[:], bfc[:])
                for k0 in range(0, N_BF, 2):
                    nc.sync.dma_start(
                        WT_sb[:, k0:k0 + 2, :],
                        WT[k0 * P:(k0 + 2) * P, :]
                        .rearrange("(j p) n -> p j n", p=P))
                for k0 in range(0, N_E3M4, 4):
                    nc.sync.dma_start(
                        WT83_sb[:, k0:k0 + 4, :],
                        WT83[k0 * P:(k0 + 4) * P, :]
                        .rearrange("(j p) n -> p j n", p=P))
                nc.sync.dma_start(
                    WT8_sb[:],
                    WT8.rearrange("(j p) n -> p j n", p=P))

                # ============ phase A: q and beta matvecs (fp8, x1024) ======
                with tc.tile_pool(name="psA", bufs=2, space="PSUM") as psA:
                    for dc in range(DC):
                        qt = psA.tile([P, B_LOC], F32, tag="q")
                        for hc in range(HC):
                            nc.tensor.matmul(
                                qt[:],
                                f8c_sb[:, hc * D + dc * P:
                                       hc * D + (dc + 1) * P],
                                f8c_sb[:, F8C_HT + hc * B_LOC:
                                       F8C_HT + (hc + 1) * B_LOC],
                                start=(hc == 0), stop=(hc == HC - 1))
                        nc.scalar.activation(qb[:, dc, :], qt[:], AF.Identity,
                                             bias=f32c_sb[:, 16 + dc:17 + dc],
                                             scale=1.0 / S_W)
                    bps = psA.tile([1, B_LOC], F32, tag="beta", bufs=1)
                    for hc in range(HC):
                        nc.tensor.matmul(
                            bps[:],
                            f8c_sb[:, F8C_WSEL + hc:F8C_WSEL + hc + 1],
                            f8c_sb[:, F8C_HT + hc * B_LOC:
                                   F8C_HT + (hc + 1) * B_LOC],
                            start=(hc == 0), stop=(hc == HC - 1))
                    # beta = sigmoid(z) = 0.5*tanh(z/2) + 0.5 (one ACT table)
                    nc.scalar.activation(tb[:], bps[:], AF.Tanh,
                                         bias=f32c_sb[0:1, 20:21],
                                         scale=0.5 / S_W)
                    # beta' = 64*sigmoid(z): the 64 is the fp8 fc x-scale
                    nc.vector.tensor_scalar(beta_sb[:], tb[:], 32.0, 32.0,
                                            op0=ALU.mult, op1=ALU.add)

                # ================= phase B: attention + gates + LSTM ========
                with tc.tile_pool(name="pslog", bufs=2, space="PSUM") as pslog, \
                     tc.tile_pool(name="pssum", bufs=1, space="PSUM") as pssum, \
                     tc.tile_pool(name="psctx", bufs=2, space="PSUM") as psctx, \
                     tc.tile_pool(name="psg", bufs=1, space="PSUM") as psg:

                    # gates psum [128(g), gc*4+b]; single start=True via the
                    # K=16 bias-broadcast preload matmul
                    g_ps = psg.tile([P, GC * B_LOC], F32, tag="g")
                    nc.tensor.matmul(g_ps[:], smallc_sb[:, 0:P],
                                     smallc_sb[:, SC_BRHS:SC_BRHS + GC * B_LOC],
                                     start=True, stop=False)

                    fcA = {}
                    for b in range(B_LOC):
                        for h in range(2):
                            u = b * 2 + h
                            r = u
                            projt, featst = projts[u], featsts[u]

                            # relu(proj + q) on ACT, bf16 out
                            hatt = hattp.tile([P, DC, HALF], BF16)
                            for dc in range(DC):
                                nc.scalar.activation(
                                    hatt[:, dc, :], projt[:, dc, :], AF.Relu,
                                    bias=qb[:, dc, b:b + 1])

                            # mask preload -> logitsT psum [128, 8], one start
                            lg_ps = pslog.tile([P, LC], F32)
                            nc.tensor.matmul(
                                lg_ps[:],
                                smallc_sb[0:LC, SC_MADD + r * P:
                                          SC_MADD + (r + 1) * P],
                                smallc_sb[0:LC, SC_ID8:SC_ID8 + LC],
                                start=True, stop=False)
                            # logitsT: stationary hatt chunks, moving w column
                            for dc in range(DC):
                                for lc in range(LC):
                                    nc.tensor.matmul(
                                        lg_ps[:, lc:lc + 1],
                                        hatt[:, dc, lc * P:(lc + 1) * P],
                                        bfc_sb[:, BFC_WPF + dc * 2 + h:
                                               BFC_WPF + dc * 2 + h + 1],
                                        start=False,
                                        stop=(dc == DC - 1 and lc == LC - 1))

                            # softmax: exp on [128, 8]; alpha straight to fp8
                            alpha_t = alphap.tile([P, LC, 1], F8)
                            nc.scalar.activation(alpha_t[:, :, 0], lg_ps[:],
                                                 AF.Exp)
                            sums_ps = pssum.tile([1, LC], F32, tag="sums")
                            nc.tensor.matmul(sums_ps[:], ones_f8[:],
                                             alpha_t[:, :, 0],
                                             start=True, stop=True)
                            nc.vector.tensor_reduce(
                                sums_sb[0:1, r:r + 1], sums_ps[0:1, :],
                                axis=mybir.AxisListType.X, op=ALU.add)
                            nc.vector.reciprocal(recips[0:1, r:r + 1],
                                                 sums_sb[0:1, r:r + 1])
                            nc.vector.tensor_tensor(svals[0:1, r:r + 1],
                                                    recips[0:1, r:r + 1],
                                                    beta_sb[0:1, b:b + 1],
                                                    op=ALU.mult)

                            # ctx[1, 512] += alpha_lc.T @ feats_lc (fp8)
                            ctx_ps = psctx.tile([1, D], F32)
                            for lc in range(LC):
                                nc.tensor.matmul(
                                    ctx_ps[:], alpha_t[:, lc, :],
                                    featst[:, lc, :],
                                    start=(lc == 0), stop=(lc == LC - 1))
                            if h == 0:
                                fcA_b = fcp.tile([1, D], F32, tag="fcA", bufs=2)
                                nc.vector.tensor_scalar_mul(
                                    fcA_b[:], ctx_ps[0:1, :],
                                    svals[0:1, r:r + 1])
                                fcA[b] = fcA_b
                            else:
                                fc_b = fcp.tile([1, D], F32, tag="fcB", bufs=2)
                                nc.vector.scalar_tensor_tensor(
                                    fc_b[:], ctx_ps[0:1, :], svals[0:1, r:r + 1],
                                    fcA[b][:], op0=ALU.mult, op1=ALU.add)
                                for dc in range(DC):
                                    tr_ps = pssum.tile([P, 1], F32, tag="tr",
                                                       bufs=2)
                                    nc.tensor.transpose(
                                        tr_ps[:, 0:1],
                                        fc_b[0:1, dc * P:(dc + 1) * P],
                                        ident[0:1, 0:1])
                                    nc.vector.tensor_copy(
                                        xf8_sb[:, dc * B_LOC + b:
                                               dc * B_LOC + b + 1],
                                        tr_ps[:])

                    # ========== phase C: gates + LSTM tail ==================
                    # gates matmuls in WT-arrival order; fp8 fc chunks (and
                    # the psum stop) land last, as the final W bytes arrive
                    for kc in range(N_BF):
                        for gc in range(GC):
                            nc.tensor.matmul(
                                g_ps[:, gc * B_LOC:(gc + 1) * B_LOC],
                                WT_sb[:, kc, gc * P:(gc + 1) * P],
                                bfc_sb[:, kc * B_LOC:(kc + 1) * B_LOC],
                                start=False, stop=False)
                    for j in range(N_E3M4):
                        kc = N_BF + j
                        for gc in range(GC):
                            nc.tensor.matmul(
                                g_ps[:, gc * B_LOC:(gc + 1) * B_LOC],
                                WT83_sb[:, j, gc * P:(gc + 1) * P],
                                bfc_sb[:, kc * B_LOC:(kc + 1) * B_LOC],
                                start=False, stop=False)
                    for dc in range(4):
                        for gc in range(GC):
                            nc.tensor.matmul(
                                g_ps[:, gc * B_LOC:(gc + 1) * B_LOC],
                                WT8_sb[:, dc, gc * P:(gc + 1) * P],
                                xf8_sb[:, dc * B_LOC:(dc + 1) * B_LOC],
                                start=False,
                                stop=(dc == 3 and gc == GC - 1))

                    # transposed LSTM tail; gate rows (permuted) = [i,f,o,g]
                    lstm = const
                    t_if = lstm.tile([P, 32], F32)
                    t_o = lstm.tile([P, 16], F32)
                    t_g = lstm.tile([P, 16], F32)
                    t2 = lstm.tile([P, 16], F32)
                    c2 = lstm.tile([P, 16], F32)
                    outt = lstm.tile([P, 32], F32)   # [c_new | h_new]

                    # sigmoid(x) = 0.5*tanh(x/2)+0.5 for i, f, o; tanh
                    # for g; psum carries S_G=256-scaled gates
                    nc.scalar.activation(t_if[:], g_ps[:, 0:32], AF.Tanh,
                                         scale=0.5 / S_G)
                    nc.scalar.activation(t_g[:], g_ps[:, 48:64], AF.Tanh,
                                         scale=1.0 / S_G)
                    nc.scalar.activation(t_o[:], g_ps[:, 32:48], AF.Tanh,
                                         scale=0.5 / S_G)
                    nc.vector.tensor_scalar(t_if[:], t_if[:], 0.5, 0.5,
                                            op0=ALU.mult, op1=ALU.add)
                    # c_new = sig_f * c_last + sig_i * tanh_g
                    nc.vector.tensor_tensor(outt[:, 0:16], t_if[:, 16:32],
                                            f32c_sb[:, 0:16], op=ALU.mult)
                    nc.vector.tensor_tensor(c2[:], t_if[:, 0:16], t_g[:],
                                            op=ALU.mult)
                    nc.vector.tensor_tensor(outt[:, 0:16], outt[:, 0:16],
                                            c2[:], op=ALU.add)
                    nc.vector.tensor_scalar(t_o[:], t_o[:], 0.5, 0.5,
                                            op0=ALU.mult, op1=ALU.add)
                    nc.scalar.activation(t2[:], outt[:, 0:16], AF.Tanh)
                    nc.vector.tensor_tensor(outt[:, 16:32], t_o[:], t2[:],
                                            op=ALU.mult)
                    nc.sync.dma_start(out_d[:], outt[:])

    nc.compile()
    return nc


_NC_CACHE = None


def _get_nc():
    global _NC_CACHE
    if _NC_CACHE is None:
        _NC_CACHE = build_nc()
    return _NC_CACHE


def split_out(arr):
    """[128, 32] device layout -> (h_new [B_LOC, H], c_new [B_LOC, H])."""
    a = np.asarray(arr, np.float32).reshape(P, 2, HC, B_LOC)
    c = np.ascontiguousarray(a[:, 0].transpose(2, 1, 0).reshape(B_LOC, H))
    h = np.ascontiguousarray(a[:, 1].transpose(2, 1, 0).reshape(B_LOC, H))
    return h, c


def make_in_maps(features, features_proj, hidden_states, cell_states,
                 caption_hidden_states, w_h2a, b_h2a, w_patt, b_patt,
                 w_fatt, b_fatt, w_sel, b_sel, w_ih, w_hh, b_ih, b_hh,
                 mask, feature_idx):
    assert int(feature_idx) == FIDX
    import ml_dtypes
    f32 = np.float32
    bf16 = ml_dtypes.bfloat16
    f8 = ml_dtypes.float8_e4m3
    features = np.asarray(features, f32)
    features_proj = np.asarray(features_proj, f32)
    h_last = np.asarray(hidden_states, f32)[-1]          # [B, H]
    c_last = np.asarray(cell_states, f32)[-1]            # [B, H]
    cap = np.asarray(caption_hidden_states, f32)         # [B, H]
    mask = np.asarray(mask)

    # shared (replicated) tensors — layout-only host prep + dtype casts
    Wfull = np.concatenate([np.asarray(w_ih, f32), np.asarray(w_hh, f32)], axis=1)
    gate_perm = np.r_[0:512, 512:1024, 1536:2048, 1024:1536]   # [i, f, o, g]
    k_perm = np.r_[0:512, 1024:1536, 1536:2048, 512:1024]      # [cap,feat,h,fc]
    f83 = ml_dtypes.float8_e3m4
    b_ihh = (np.asarray(b_ih, f32) + np.asarray(b_hh, f32))[gate_perm] * S_G
    WTf = Wfull[gate_perm][:, k_perm].T                       # [k, g]
    WTh = np.ascontiguousarray(WTf[0:N_BF * P] * S_G).astype(bf16)
    WT83h = np.ascontiguousarray(
        WTf[N_BF * P:12 * P] * S_G).astype(f83)
    WT8h = np.ascontiguousarray(WTf[12 * P:] * (S_G / 64.0)).astype(f8)

    # small bf16 pack rows=16: biasT2 | brhs | ident8 | madd2 (per core)
    sc_base = np.zeros((16, 1224), f32)
    sc_base[:, 0:P] = b_ihh.reshape(GC, P)
    sc_base[:, SC_BRHS:SC_BRHS + GC * B_LOC] = np.kron(
        np.eye(GC, dtype=f32), np.ones((1, B_LOC), f32))
    sc_base[0:LC, SC_ID8:SC_ID8 + LC] = np.eye(LC, dtype=f32)

    # f8 pack: w_h2a (hc-major) | w_sel | hT8 (hT8 filled per core)
    f8c_w = np.zeros((P, 2068), f32)
    w_h2aT = np.asarray(w_h2a, f32).T * S_W                    # [H, D]
    f8c_w[:, 0:2048] = w_h2aT.reshape(HC, P, D).transpose(1, 0, 2) \
        .reshape(P, 2048)
    f8c_w[:, F8C_WSEL:F8C_WSEL + HC] = (
        np.asarray(w_sel, f32).T * S_W).reshape(HC, P).T

    w_pf = np.stack([np.asarray(w_patt, f32)[0], np.asarray(w_fatt, f32)[0]],
                    axis=1)                                    # [D, 2]
    madd = np.where(mask, f32(0), f32(-1e30))                  # [B, L]

    in_maps = []
    for c in range(N_CORES):
        sl = slice(c * B_LOC, (c + 1) * B_LOC)
        sc = sc_base.copy()
        # madd2[lc, (b*2+h)*128 + p] = madd_loc[b, h*HALF + lc*128 + p]
        m = madd[sl].reshape(B_LOC, 2, LC, P)
        sc[0:LC, SC_MADD:SC_MADD + 2 * B_LOC * P] = (
            m.transpose(2, 0, 1, 3).reshape(LC, 2 * B_LOC * P))

        f8cc = f8c_w.copy()
        f8cc[:, F8C_HT:F8C_HT + 16] = (
            h_last[sl].T.reshape(HC, P, B_LOC).transpose(1, 0, 2)
            .reshape(P, 16))

        bfc = np.zeros((P, 72), f32)
        xh = np.stack([cap[sl], features[sl, FIDX, :], h_last[sl]], axis=0)
        # xh [3, B_LOC, H] -> bfc[p, kc*4+b] for kc blocks cap/feat/h
        bfc[:, 0:48] = xh.reshape(3, B_LOC, HC, P).transpose(3, 0, 2, 1) \
            .reshape(P, 48)
        bfc[:, BFC_WPF:BFC_WPF + 8] = w_pf.reshape(DC, P, 2) \
            .transpose(1, 0, 2).reshape(P, 8)

        f32cc = np.zeros((P, 21), f32)
        f32cc[:, 0:16] = c_last[sl].T.reshape(HC, P, B_LOC) \
            .transpose(1, 0, 2).reshape(P, 16)
        f32cc[:, 16:20] = np.asarray(b_h2a, f32).reshape(DC, P).T
        f32cc[0, 20] = np.asarray(b_sel, f32).reshape(-1)[0] * 0.5

        in_maps.append({
            "projT": np.ascontiguousarray(
                features_proj[sl].transpose(0, 2, 1)).astype(f8),
            "feats": np.ascontiguousarray(features[sl]).astype(f8),
            "WT": WTh,
            "WT83": WT83h,
            "WT8": WT8h,
            "f8c": f8cc.astype(f8),
            "f32c": f32cc,
            "bfc": bfc.astype(bf16),
            "smallc": sc.astype(bf16),
        })
    return in_maps


def run(trace=False, **inputs):
    nc = _get_nc()
    in_maps = make_in_maps(**inputs)
    res = run_bass_kernel_spmd(nc, in_maps, core_ids=list(range(N_CORES)),
                               trace=trace)
    hs, cs = [], []
    for c in range(N_CORES):
        h, cc = split_out(res.results[c]["out"])
        hs.append(h)
        cs.append(cc)
    return (np.concatenate(hs)[None], np.concatenate(cs)[None]), res


def kernel(**inputs):
    out, _ = run(trace=False, **inputs)
    return out


# revision 10
# speedup vs baseline: 2.0261x; 1.0840x over previous
"""EventRNN (sparse_attention) Trainium2 Bass kernel — v3.

Full-input contract: kernel(**inputs) takes the complete arrays from
setup_inputs() and returns the full (h_new[None], c_new[None]) tuple.

Sharding: data-parallel over batch B=32 across 8 NeuronCores (4 batches
per core); all weights replicated. Host-side prep is layout-only
(transposes / slicing / dtype casts / bool->additive mask).

Design (DMA-roofline focused):
 - features / features_proj / w_h2a / w_sel / h(for matvecs) ship as
   fp8e4m3 (validated on host: final rel err ~2.4e-3 vs 2.7e-3 for the
   all-bf16 baseline). LSTM W stays bf16 (fp8 W alone costs 3.5e-2).
   Per-core DMA drops 25.8 MB -> 17.1 MB.
 - DMA order: packed consts (4 DMAs) -> all proj/feats (the attention
   pipeline's food) -> LSTM W chunk pairs, fc-fed chunks last. The 2KB
   psum-bank accumulation runs under single start=True preloads (bias /
   mask matmuls) so 272 gate matmuls + 256 logits matmuls can land in
   shared banks.
 - logits are computed TRANSPOSED: lhsT = 128x128 relu'd-proj chunks
   (stationary), rhs = attention weight column -> logits in PSUM as
   [128(l), 8(lc)]. Softmax exp runs on ACT at free-size 8 (128x fewer
   cycles than a [1,1024] row), alpha needs no PE transposes, no
   max-subtract (bounded logits, fp32 psum), alpha written directly as
   fp8 for the ctx matvec.
 - gates matmul runs output-transposed: lhsT = W.T 128x128 chunks
   (stationary), rhs = xhT [128,4]; psum [128(g), 64]. LSTM elementwise
   tail in the transposed layout; host un-transposes [128, 32] output.
 - one ACT table for the whole kernel (exp_and_others: relu/exp/
   identity/tanh); sigmoids via 0.5*tanh(x/2)+0.5. No table switches.
"""

import numpy as np

import concourse.bacc as bacc
import concourse.mybir as mybir
import concourse.tile as tile
import concourse.masks as masks
from concourse.bass_utils import run_bass_kernel_spmd

F32 = mybir.dt.float32
BF16 = mybir.dt.bfloat16
F8 = mybir.dt.float8e4
F83 = mybir.dt.float8e3
AF = mybir.ActivationFunctionType
ALU = mybir.AluOpType

B, L, D, H = 32, 2048, 512, 512
N_CORES = 8
B_LOC = B // N_CORES          # 4 batches per core
FIDX = 1024                   # static feature_idx from setup_inputs()
HALF = L // 2                 # past/future split == 1024
P = 128
DC = D // P                   # 4 d-chunks
HC = H // P                   # 4 h-chunks
LC = HALF // P                # 8 l-chunks per half
KC = 16                       # k-chunks of the fused LSTM matmul (2048/128)
GC = 16                       # gate-column chunks (4H/128)
G4 = 4 * H
S_W = 1024.0                  # fp8 scale for the small matvec weights
S_G = 128.0                   # global gates scale (W*128, descaled in ACT)
N_E3M4 = 8                    # trailing cap/feat/h W chunks in fp8e3m4
N_BF = 12 - N_E3M4            # leading W chunks kept bf16
# xhT k-chunk order: [cap 0:4 | feature 4:8 | h 8:12 | fc 12:16] so the
# attention-context-dependent chunks come last (their W pair is also the
# last DMA; the gates tail then runs right as the final W bytes land).
FC_KCS = (12, 13, 14, 15)
WT_PAIRS = [(0, 1), (2, 3), (4, 5), (6, 7), (8, 9), (10, 11), (12, 13),
            (14, 15)]
# packed-const offsets
F8C_WSEL = 2048               # f8 pack: w_h2a [0:2048], w_sel, hT8
F8C_HT = 2052
BFC_WPF = 64                  # bf16 pack: xhT [0:64], w_pf [64:72]
SC_BRHS = 128                 # small pack rows=16: biasT2 [0:128], brhs,
SC_ID8 = 192                  # ident8 [192:200], madd2 [200:1224] (rows 0:8)
SC_MADD = 200


def build_nc():
    nc = bacc.Bacc("TRN2", target_bir_lowering=False, debug=False,
                   num_devices=N_CORES)

    # ---- DRAM I/O ----
    projT = nc.dram_tensor("projT", [B_LOC, D, L], F8, kind="ExternalInput").ap()
    feats = nc.dram_tensor("feats", [B_LOC, L, D], F8, kind="ExternalInput").ap()
    WT = nc.dram_tensor("WT", [N_BF * P, G4], BF16, kind="ExternalInput").ap()
    WT83 = nc.dram_tensor("WT83", [N_E3M4 * P, G4], F83,
                          kind="ExternalInput").ap()
    WT8 = nc.dram_tensor("WT8", [4 * P, G4], F8, kind="ExternalInput").ap()
    f8c = nc.dram_tensor("f8c", [P, 2068], F8, kind="ExternalInput").ap()
    f32c = nc.dram_tensor("f32c", [P, 21], F32, kind="ExternalInput").ap()
    bfc = nc.dram_tensor("bfc", [P, 72], BF16, kind="ExternalInput").ap()
    smallc = nc.dram_tensor("smallc", [16, 1224], BF16, kind="ExternalInput").ap()
    out_d = nc.dram_tensor("out", [P, 32], F32, kind="ExternalOutput").ap()

    with tile.TileContext(nc) as tc:
        with tc.tile_pool(name="const", bufs=1) as const, \
             tc.tile_pool(name="wres", bufs=1) as wres:
            # ---- resident constants / packed small inputs ----
            ident = const.tile([P, P], F32)
            masks.make_identity(nc, ident[:])
            ones_f8 = const.tile([P, 1], F8)
            nc.gpsimd.memset(ones_f8[:], 1.0)

            f8c_sb = const.tile([P, 2068], F8)
            f32c_sb = const.tile([P, 21], F32)
            smallc_sb = const.tile([16, 1224], BF16)
            # xhT lives inside the bf16 pack; fc x-chunks go to the fp8 tile
            bfc_sb = const.tile([P, 72], BF16)
            xf8_sb = const.tile([P, 16], F8)

            # resident LSTM weights: bf16 + e3m4 cap/feat/h + e4m3 fc
            WT_sb = wres.tile([P, N_BF, G4], BF16)
            WT83_sb = wres.tile([P, N_E3M4, G4], F83)
            WT8_sb = wres.tile([P, 4, G4], F8)

            # scalars along free dims, r = b*2 + h
            qb = const.tile([P, DC, B_LOC], F32)
            tb = const.tile([1, B_LOC], F32)
            beta_sb = const.tile([1, B_LOC], F32)
            sums_sb = const.tile([1, 2 * B_LOC], F32)
            recips = const.tile([1, 2 * B_LOC], F32)
            svals = const.tile([1, 2 * B_LOC], F32)

            with tc.tile_pool(name="proj", bufs=8) as projp, \
                 tc.tile_pool(name="hatt", bufs=2) as hattp, \
                 tc.tile_pool(name="fpool", bufs=8) as fpool, \
                 tc.tile_pool(name="alphap", bufs=3) as alphap, \
                 tc.tile_pool(name="fcp", bufs=2) as fcp:

                # ---- DMA order: proj0, packed consts, then the stream;
                # bf16 W pairs next-to-last, the fp8 fc W chunk dead last ----
                projts, featsts = [], []
                for u in range(2 * B_LOC):
                    b, h = divmod(u, 2)
                    projt = projp.tile([P, DC, HALF], F8)
                    nc.sync.dma_start(
                        projt[:],
                        projT[b, :, h * HALF:(h + 1) * HALF]
                        .rearrange("(c p) l -> p c l", p=P))
                    projts.append(projt)
                    if u == 0:
                        nc.sync.dma_start(f8c_sb[:], f8c[:])
                        nc.sync.dma_start(f32c_sb[:], f32c[:])
                        nc.sync.dma_start(smallc_sb[:], smallc[:])
                    featst = fpool.tile([P, LC, D], F8)
                    nc.sync.dma_start(
                        featst[:],
                        feats[b, h * HALF:(h + 1) * HALF, :]
                        .rearrange("(c p) d -> p c d", p=P))
                    featsts.append(featst)
                    if u == 0:
                        nc.sync.dma_start(bfc_sb[:], bfc[:])
                for k0 in range(0, N_BF, 2):
                    nc.sync.dma_start(
                        WT_sb[:, k0:k0 + 2, :],
                        WT[k0 * P:(k0 + 2) * P, :]
                        .rearrange("(j p) n -> p j n", p=P))
                for k0 in range(0, N_E3M4, 4):
                    nc.sync.dma_start(
                        WT83_sb[:, k0:k0 + 4, :],
                        WT83[k0 * P:(k0 + 4) * P, :]
                        .rearrange("(j p) n -> p j n", p=P))
                nc.sync.dma_start(
                    WT8_sb[:],
                    WT8.rearrange("(j p) n -> p j n", p=P))

                # ============ phase A: q and beta matvecs (fp8, x1024) ======
                with tc.tile_pool(name="psA", bufs=2, space="PSUM") as psA:
                    for dc in range(DC):
                        qt = psA.tile([P, B_LOC], F32, tag="q")
                        for hc in range(HC):
                            nc.tensor.matmul(
                                qt[:],
                                f8c_sb[:, hc * D + dc * P:
                                       hc * D + (dc + 1) * P],
                                f8c_sb[:, F8C_HT + hc * B_LOC:
                                       F8C_HT + (hc + 1) * B_LOC],
                                start=(hc == 0), stop=(hc == HC - 1))
                        nc.scalar.activation(qb[:, dc, :], qt[:], AF.Identity,
                                             bias=f32c_sb[:, 16 + dc:17 + dc],
                                             scale=1.0 / S_W)
                    bps = psA.tile([1, B_LOC], F32, tag="beta", bufs=1)
                    for hc in range(HC):
                        nc.tensor.matmul(
                            bps[:],
                            f8c_sb[:, F8C_WSEL + hc:F8C_WSEL + hc + 1],
                            f8c_sb[:, F8C_HT + hc * B_LOC:
                                   F8C_HT + (hc + 1) * B_LOC],
                            start=(hc == 0), stop=(hc == HC - 1))
                    # beta = sigmoid(z) = 0.5*tanh(z/2) + 0.5 (one ACT table)
                    nc.scalar.activation(tb[:], bps[:], AF.Tanh,
                                         bias=f32c_sb[0:1, 20:21],
                                         scale=0.5 / S_W)
                    # beta' = 64*sigmoid(z): the 64 is the fp8 fc x-scale
                    nc.vector.tensor_scalar(beta_sb[:], tb[:], 32.0, 32.0,
                                            op0=ALU.mult, op1=ALU.add)

                # ================= phase B: attention + gates + LSTM ========
                with tc.tile_pool(name="pslog", bufs=2, space="PSUM") as pslog, \
                     tc.tile_pool(name="pssum", bufs=1, space="PSUM") as pssum, \
                     tc.tile_pool(name="psctx", bufs=2, space="PSUM") as psctx, \
                     tc.tile_pool(name="psg", bufs=1, space="PSUM") as psg:

                    # gates psum [128(g), gc*4+b]; single start=True via the
                    # K=16 bias-broadcast preload matmul
                    g_ps = psg.tile([P, GC * B_LOC], F32, tag="g")
                    nc.tensor.matmul(g_ps[:], smallc_sb[:, 0:P],
                                     smallc_sb[:, SC_BRHS:SC_BRHS + GC * B_LOC],
                                     start=True, stop=False)

                    fcA = {}
                    for b in range(B_LOC):
                        for h in range(2):
                            u = b * 2 + h
                            r = u
                            projt, featst = projts[u], featsts[u]

                            # relu(proj + q): dc0 on Pool, dc1 on DVE,
                            # dc2/dc3 on ACT (three engines in parallel)
                            hatt = hattp.tile([P, DC, HALF], BF16)
                            nc.gpsimd.tensor_scalar(
                                hatt[:, 0, :], projt[:, 0, :],
                                qb[:, 0, b:b + 1], 0.0,
                                op0=ALU.add, op1=ALU.max)
                            nc.vector.tensor_scalar(
                                hatt[:, 1, :], projt[:, 1, :],
                                qb[:, 1, b:b + 1], 0.0,
                                op0=ALU.add, op1=ALU.max)
                            for dc in (2, 3):
                                nc.scalar.activation(
                                    hatt[:, dc, :], projt[:, dc, :], AF.Relu,
                                    bias=qb[:, dc, b:b + 1])

                            # mask preload -> logitsT psum [128, 8], one start
                            lg_ps = pslog.tile([P, LC], F32)
                            nc.tensor.matmul(
                                lg_ps[:],
                                smallc_sb[0:LC, SC_MADD + r * P:
                                          SC_MADD + (r + 1) * P],
                                smallc_sb[0:LC, SC_ID8:SC_ID8 + LC],
                                start=True, stop=False)
                            # logitsT: stationary hatt chunks, moving w column
                            for dc in range(DC):
                                for lc in range(LC):
                                    nc.tensor.matmul(
                                        lg_ps[:, lc:lc + 1],
                                        hatt[:, dc, lc * P:(lc + 1) * P],
                                        bfc_sb[:, BFC_WPF + dc * 2 + h:
                                               BFC_WPF + dc * 2 + h + 1],
                                        start=False,
                                        stop=(dc == DC - 1 and lc == LC - 1))

                            # softmax: exp on [128, 8]; alpha straight to fp8
                            alpha_t = alphap.tile([P, LC, 1], F8)
                            nc.scalar.activation(alpha_t[:, :, 0], lg_ps[:],
                                                 AF.Exp)
                            sums_ps = pssum.tile([1, LC], F32, tag="sums")
                            nc.tensor.matmul(sums_ps[:], ones_f8[:],
                                             alpha_t[:, :, 0],
                                             start=True, stop=True)
                            nc.vector.tensor_reduce(
                                sums_sb[0:1, r:r + 1], sums_ps[0:1, :],
                                axis=mybir.AxisListType.X, op=ALU.add)
                            nc.vector.reciprocal(recips[0:1, r:r + 1],
                                                 sums_sb[0:1, r:r + 1])
                            nc.vector.tensor_tensor(svals[0:1, r:r + 1],
                                                    recips[0:1, r:r + 1],
                                                    beta_sb[0:1, b:b + 1],
                                                    op=ALU.mult)

                            # ctx[1, 512] += alpha_lc.T @ feats_lc (fp8)
                            ctx_ps = psctx.tile([1, D], F32)
                            for lc in range(LC):
                                nc.tensor.matmul(
                                    ctx_ps[:], alpha_t[:, lc, :],
                                    featst[:, lc, :],
                                    start=(lc == 0), stop=(lc == LC - 1))
                            if h == 0:
                                fcA_b = fcp.tile([1, D], F32, tag="fcA", bufs=2)
                                nc.vector.tensor_scalar_mul(
                                    fcA_b[:], ctx_ps[0:1, :],
                                    svals[0:1, r:r + 1])
                                fcA[b] = fcA_b
                            else:
                                fc_b = fcp.tile([1, D], F32, tag="fcB", bufs=2)
                                nc.vector.scalar_tensor_tensor(
                                    fc_b[:], ctx_ps[0:1, :], svals[0:1, r:r + 1],
                                    fcA[b][:], op0=ALU.mult, op1=ALU.add)
                                for dc in range(DC):
                                    tr_ps = pssum.tile([P, 1], F32, tag="tr",
                                                       bufs=2)
                                    nc.tensor.transpose(
                                        tr_ps[:, 0:1],
                                        fc_b[0:1, dc * P:(dc + 1) * P],
                                        ident[0:1, 0:1])
                                    nc.vector.tensor_copy(
                                        xf8_sb[:, dc * B_LOC + b:
                                               dc * B_LOC + b + 1],
                                        tr_ps[:])

                    # ========== phase C: gates + LSTM tail ==================
                    # gates matmuls in WT-arrival order; fp8 fc chunks (and
                    # the psum stop) land last, as the final W bytes arrive
                    for kc in range(N_BF):
                        for gc in range(GC):
                            nc.tensor.matmul(
                                g_ps[:, gc * B_LOC:(gc + 1) * B_LOC],
                                WT_sb[:, kc, gc * P:(gc + 1) * P],
                                bfc_sb[:, kc * B_LOC:(kc + 1) * B_LOC],
                                start=False, stop=False)
                    for j in range(N_E3M4):
                        kc = N_BF + j
                        for gc in range(GC):
                            nc.tensor.matmul(
                                g_ps[:, gc * B_LOC:(gc + 1) * B_LOC],
                                WT83_sb[:, j, gc * P:(gc + 1) * P],
                                bfc_sb[:, kc * B_LOC:(kc + 1) * B_LOC],
                                start=False, stop=False)
                    for dc in range(4):
                        for gc in range(GC):
                            nc.tensor.matmul(
                                g_ps[:, gc * B_LOC:(gc + 1) * B_LOC],
                                WT8_sb[:, dc, gc * P:(gc + 1) * P],
                                xf8_sb[:, dc * B_LOC:(dc + 1) * B_LOC],
                                start=False,
                                stop=(dc == 3 and gc == GC - 1))

                    # transposed LSTM tail; gate rows (permuted) = [i,f,o,g]
                    lstm = const
                    t_if = lstm.tile([P, 32], F32)
                    t_o = lstm.tile([P, 16], F32)
                    t_g = lstm.tile([P, 16], F32)
                    t2 = lstm.tile([P, 16], F32)
                    c2 = lstm.tile([P, 16], F32)
                    outt = lstm.tile([P, 32], F32)   # [c_new | h_new]

                    # sigmoid(x) = 0.5*tanh(x/2)+0.5 for i, f, o; tanh
                    # for g; psum carries S_G=256-scaled gates
                    nc.scalar.activation(t_if[:], g_ps[:, 0:32], AF.Tanh,
                                         scale=0.5 / S_G)
                    nc.scalar.activation(t_g[:], g_ps[:, 48:64], AF.Tanh,
                                         scale=1.0 / S_G)
                    nc.scalar.activation(t_o[:], g_ps[:, 32:48], AF.Tanh,
                                         scale=0.5 / S_G)
                    nc.vector.tensor_scalar(t_if[:], t_if[:], 0.5, 0.5,
                                            op0=ALU.mult, op1=ALU.add)
                    # c_new = sig_f * c_last + sig_i * tanh_g
                    nc.vector.tensor_tensor(outt[:, 0:16], t_if[:, 16:32],
                                            f32c_sb[:, 0:16], op=ALU.mult)
                    nc.vector.tensor_tensor(c2[:], t_if[:, 0:16], t_g[:],
                                            op=ALU.mult)
                    nc.vector.tensor_tensor(outt[:, 0:16], outt[:, 0:16],
                                            c2[:], op=ALU.add)
                    nc.vector.tensor_scalar(t_o[:], t_o[:], 0.5, 0.5,
                                            op0=ALU.mult, op1=ALU.add)
                    nc.scalar.activation(t2[:], outt[:, 0:16], AF.Tanh)
                    nc.vector.tensor_tensor(outt[:, 16:32], t_o[:], t2[:],
                                            op=ALU.mult)
                    nc.sync.dma_start(out_d[:], outt[:])

    nc.compile()
    return nc


_NC_CACHE = None


def _get_nc():
    global _NC_CACHE
    if _NC_CACHE is None:
        _NC_CACHE = build_nc()
    return _NC_CACHE


def split_out(arr):
    """[128, 32] device layout -> (h_new [B_LOC, H], c_new [B_LOC, H])."""
    a = np.asarray(arr, np.float32).reshape(P, 2, HC, B_LOC)
    c = np.ascontiguousarray(a[:, 0].transpose(2, 1, 0).reshape(B_LOC, H))
    h = np.ascontiguousarray(a[:, 1].transpose(2, 1, 0).reshape(B_LOC, H))
    return h, c


def make_in_maps(features, features_proj, hidden_states, cell_states,
                 caption_hidden_states, w_h2a, b_h2a, w_patt, b_patt,
                 w_fatt, b_fatt, w_sel, b_sel, w_ih, w_hh, b_ih, b_hh,
                 mask, feature_idx):
    assert int(feature_idx) == FIDX
    import ml_dtypes
    f32 = np.float32
    bf16 = ml_dtypes.bfloat16
    f8 = ml_dtypes.float8_e4m3
    features = np.asarray(features, f32)
    features_proj = np.asarray(features_proj, f32)
    h_last = np.asarray(hidden_states, f32)[-1]          # [B, H]
    c_last = np.asarray(cell_states, f32)[-1]            # [B, H]
    cap = np.asarray(caption_hidden_states, f32)         # [B, H]
    mask = np.asarray(mask)

    # shared (replicated) tensors — layout-only host prep + dtype casts
    Wfull = np.concatenate([np.asarray(w_ih, f32), np.asarray(w_hh, f32)], axis=1)
    gate_perm = np.r_[0:512, 512:1024, 1536:2048, 1024:1536]   # [i, f, o, g]
    k_perm = np.r_[0:512, 1024:1536, 1536:2048, 512:1024]      # [cap,feat,h,fc]
    f83 = ml_dtypes.float8_e3m4
    b_ihh = (np.asarray(b_ih, f32) + np.asarray(b_hh, f32))[gate_perm] * S_G
    WTf = Wfull[gate_perm][:, k_perm].T                       # [k, g]
    WTh = np.ascontiguousarray(WTf[0:N_BF * P] * S_G).astype(bf16)
    WT83h = np.ascontiguousarray(
        WTf[N_BF * P:12 * P] * S_G).astype(f83)
    WT8h = np.ascontiguousarray(WTf[12 * P:] * (S_G / 64.0)).astype(f8)

    # small bf16 pack rows=16: biasT2 | brhs | ident8 | madd2 (per core)
    sc_base = np.zeros((16, 1224), f32)
    sc_base[:, 0:P] = b_ihh.reshape(GC, P)
    sc_base[:, SC_BRHS:SC_BRHS + GC * B_LOC] = np.kron(
        np.eye(GC, dtype=f32), np.ones((1, B_LOC), f32))
    sc_base[0:LC, SC_ID8:SC_ID8 + LC] = np.eye(LC, dtype=f32)

    # f8 pack: w_h2a (hc-major) | w_sel | hT8 (hT8 filled per core)
    f8c_w = np.zeros((P, 2068), f32)
    w_h2aT = np.asarray(w_h2a, f32).T * S_W                    # [H, D]
    f8c_w[:, 0:2048] = w_h2aT.reshape(HC, P, D).transpose(1, 0, 2) \
        .reshape(P, 2048)
    f8c_w[:, F8C_WSEL:F8C_WSEL + HC] = (
        np.asarray(w_sel, f32).T * S_W).reshape(HC, P).T

    w_pf = np.stack([np.asarray(w_patt, f32)[0], np.asarray(w_fatt, f32)[0]],
                    axis=1)                                    # [D, 2]
    madd = np.where(mask, f32(0), f32(-1e30))                  # [B, L]

    in_maps = []
    for c in range(N_CORES):
        sl = slice(c * B_LOC, (c + 1) * B_LOC)
        sc = sc_base.copy()
        # madd2[lc, (b*2+h)*128 + p] = madd_loc[b, h*HALF + lc*128 + p]
        m = madd[sl].reshape(B_LOC, 2, LC, P)
        sc[0:LC, SC_MADD:SC_MADD + 2 * B_LOC * P] = (
            m.transpose(2, 0, 1, 3).reshape(LC, 2 * B_LOC * P))

        f8cc = f8c_w.copy()
        f8cc[:, F8C_HT:F8C_HT + 16] = (
            h_last[sl].T.reshape(HC, P, B_LOC).transpose(1, 0, 2)
            .reshape(P, 16))

        bfc = np.zeros((P, 72), f32)
        xh = np.stack([cap[sl], features[sl, FIDX, :], h_last[sl]], axis=0)
        # xh [3, B_LOC, H] -> bfc[p, kc*4+b] for kc blocks cap/feat/h
        bfc[:, 0:48] = xh.reshape(3, B_LOC, HC, P).transpose(3, 0, 2, 1) \
            .reshape(P, 48)
        bfc[:, BFC_WPF:BFC_WPF + 8] = w_pf.reshape(DC, P, 2) \
            .transpose(1, 0, 2).reshape(P, 8)

        f32cc = np.zeros((P, 21), f32)
        f32cc[:, 0:16] = c_last[sl].T.reshape(HC, P, B_LOC) \
            .transpose(1, 0, 2).reshape(P, 16)
        f32cc[:, 16:20] = np.asarray(b_h2a, f32).reshape(DC, P).T
        f32cc[0, 20] = np.asarray(b_sel, f32).reshape(-1)[0] * 0.5

        in_maps.append({
            "projT": np.ascontiguousarray(
                features_proj[sl].transpose(0, 2, 1)).astype(f8),
            "feats": np.ascontiguousarray(features[sl]).astype(f8),
            "WT": WTh,
            "WT83": WT83h,
            "WT8": WT8h,
            "f8c": f8cc.astype(f8),
            "f32c": f32cc,
            "bfc": bfc.astype(bf16),
            "smallc": sc.astype(bf16),
        })
    return in_maps


def run(trace=False, **inputs):
    nc = _get_nc()
    in_maps = make_in_maps(**inputs)
    res = run_bass_kernel_spmd(nc, in_maps, core_ids=list(range(N_CORES)),
                               trace=trace)
    hs, cs = [], []
    for c in range(N_CORES):
        h, cc = split_out(res.results[c]["out"])
        hs.append(h)
        cs.append(cc)
    return (np.concatenate(hs)[None], np.concatenate(cs)[None]), res


def kernel(**inputs):
    out, _ = run(trace=False, **inputs)
    return out


# revision 12
# speedup vs baseline: 2.0694x; 1.0214x over previous
"""EventRNN (sparse_attention) Trainium2 Bass kernel — v3.

Full-input contract: kernel(**inputs) takes the complete arrays from
setup_inputs() and returns the full (h_new[None], c_new[None]) tuple.

Sharding: data-parallel over batch B=32 across 8 NeuronCores (4 batches
per core); all weights replicated. Host-side prep is layout-only
(transposes / slicing / dtype casts / bool->additive mask).

Design (DMA-roofline focused):
 - features / features_proj / w_h2a / w_sel / h(for matvecs) ship as
   fp8e4m3 (validated on host: final rel err ~2.4e-3 vs 2.7e-3 for the
   all-bf16 baseline). LSTM W stays bf16 (fp8 W alone costs 3.5e-2).
   Per-core DMA drops 25.8 MB -> 17.1 MB.
 - DMA order: packed consts (4 DMAs) -> all proj/feats (the attention
   pipeline's food) -> LSTM W chunk pairs, fc-fed chunks last. The 2KB
   psum-bank accumulation runs under single start=True preloads (bias /
   mask matmuls) so 272 gate matmuls + 256 logits matmuls can land in
   shared banks.
 - logits are computed TRANSPOSED: lhsT = 128x128 relu'd-proj chunks
   (stationary), rhs = attention weight column -> logits in PSUM as
   [128(l), 8(lc)]. Softmax exp runs on ACT at free-size 8 (128x fewer
   cycles than a [1,1024] row), alpha needs no PE transposes, no
   max-subtract (bounded logits, fp32 psum), alpha written directly as
   fp8 for the ctx matvec.
 - gates matmul runs output-transposed: lhsT = W.T 128x128 chunks
   (stationary), rhs = xhT [128,4]; psum [128(g), 64]. LSTM elementwise
   tail in the transposed layout; host un-transposes [128, 32] output.
 - one ACT table for the whole kernel (exp_and_others: relu/exp/
   identity/tanh); sigmoids via 0.5*tanh(x/2)+0.5. No table switches.
"""

import numpy as np

import concourse.bacc as bacc
import concourse.mybir as mybir
import concourse.tile as tile
import concourse.masks as masks
from concourse.bass_utils import run_bass_kernel_spmd

F32 = mybir.dt.float32
BF16 = mybir.dt.bfloat16
F8 = mybir.dt.float8e4
F83 = mybir.dt.float8e3
AF = mybir.ActivationFunctionType
ALU = mybir.AluOpType

B, L, D, H = 32, 2048, 512, 512
N_CORES = 8
B_LOC = B // N_CORES          # 4 batches per core
FIDX = 1024                   # static feature_idx from setup_inputs()
HALF = L // 2                 # past/future split == 1024
P = 128
DC = D // P                   # 4 d-chunks
HC = H // P                   # 4 h-chunks
LC = HALF // P                # 8 l-chunks per half
KC = 16                       # k-chunks of the fused LSTM matmul (2048/128)
GC = 16                       # gate-column chunks (4H/128)
G4 = 4 * H
S_W = 1024.0                  # fp8 scale for the small matvec weights
S_G = 128.0                   # global gates scale (W*128, descaled in ACT)
N_E3M4 = 8                    # trailing cap/feat/h W chunks in fp8e3m4
N_BF = 12 - N_E3M4            # leading W chunks kept bf16
# xhT k-chunk order: [cap 0:4 | feature 4:8 | h 8:12 | fc 12:16] so the
# attention-context-dependent chunks come last (their W pair is also the
# last DMA; the gates tail then runs right as the final W bytes land).
FC_KCS = (12, 13, 14, 15)
WT_PAIRS = [(0, 1), (2, 3), (4, 5), (6, 7), (8, 9), (10, 11), (12, 13),
            (14, 15)]
# packed-const offsets
F8C_WSEL = 2048               # f8 pack: w_h2a [0:2048], w_sel, hT8
F8C_HT = 2052
BFC_WPF = 64                  # bf16 pack: xhT [0:64], w_pf [64:72]
SC_BRHS = 128                 # small pack rows=16: biasT2 [0:128], brhs,
SC_ID8 = 192                  # ident8 [192:200], madd2 [200:1224] (rows 0:8)
SC_MADD = 200


def build_nc():
    nc = bacc.Bacc("TRN2", target_bir_lowering=False, debug=False,
                   num_devices=N_CORES)

    # ---- DRAM I/O ----
    projT = nc.dram_tensor("projT", [B_LOC, D, L], F8, kind="ExternalInput").ap()
    feats = nc.dram_tensor("feats", [B_LOC, L, D], F8, kind="ExternalInput").ap()
    WT = nc.dram_tensor("WT", [N_BF * P, G4], BF16, kind="ExternalInput").ap()
    WT83 = nc.dram_tensor("WT83", [N_E3M4 * P, G4], F83,
                          kind="ExternalInput").ap()
    WT8 = nc.dram_tensor("WT8", [4 * P, G4], F8, kind="ExternalInput").ap()
    f8c = nc.dram_tensor("f8c", [P, 2068], F8, kind="ExternalInput").ap()
    f32c = nc.dram_tensor("f32c", [P, 21], F32, kind="ExternalInput").ap()
    bfc = nc.dram_tensor("bfc", [P, 72], BF16, kind="ExternalInput").ap()
    smallc = nc.dram_tensor("smallc", [16, 1224], BF16, kind="ExternalInput").ap()
    out_d = nc.dram_tensor("out", [P, 32], F32, kind="ExternalOutput").ap()

    with tile.TileContext(nc) as tc:
        with tc.tile_pool(name="const", bufs=1) as const, \
             tc.tile_pool(name="wres", bufs=1) as wres:
            # ---- resident constants / packed small inputs ----
            ident = const.tile([P, P], F32)
            masks.make_identity(nc, ident[:])
            ones_f8 = const.tile([P, 1], F8)
            nc.gpsimd.memset(ones_f8[:], 1.0)

            f8c_sb = const.tile([P, 2068], F8)
            f32c_sb = const.tile([P, 21], F32)
            smallc_sb = const.tile([16, 1224], BF16)
            # xhT lives inside the bf16 pack; fc x-chunks go to the fp8 tile
            bfc_sb = const.tile([P, 72], BF16)
            xf8_sb = const.tile([P, 16], F8)

            # resident LSTM weights: bf16 + e3m4 cap/feat/h + e4m3 fc
            WT_sb = wres.tile([P, N_BF, G4], BF16)
            WT83_sb = wres.tile([P, N_E3M4, G4], F83)
            WT8_sb = wres.tile([P, 4, G4], F8)

            # scalars along free dims, r = b*2 + h
            qb = const.tile([P, DC, B_LOC], F32)
            tb = const.tile([1, B_LOC], F32)
            beta_sb = const.tile([1, B_LOC], F32)
            sums_sb = const.tile([1, 2 * B_LOC], F32)
            recips = const.tile([1, 2 * B_LOC], F32)
            svals = const.tile([1, 2 * B_LOC], F32)

            with tc.tile_pool(name="proj", bufs=8) as projp, \
                 tc.tile_pool(name="hatt", bufs=2) as hattp, \
                 tc.tile_pool(name="fpool", bufs=8) as fpool, \
                 tc.tile_pool(name="alphap", bufs=3) as alphap, \
                 tc.tile_pool(name="fcp", bufs=2) as fcp:

                # ---- DMA order: proj0, packed consts, then the stream;
                # bf16 W pairs next-to-last, the fp8 fc W chunk dead last ----
                projts, featsts = [], []
                for u in range(2 * B_LOC):
                    b, h = divmod(u, 2)
                    projt = projp.tile([P, DC, HALF], F8)
                    nc.sync.dma_start(
                        projt[:],
                        projT[b, :, h * HALF:(h + 1) * HALF]
                        .rearrange("(c p) l -> p c l", p=P))
                    projts.append(projt)
                    if u == 0:
                        nc.sync.dma_start(f8c_sb[:], f8c[:])
                        nc.sync.dma_start(f32c_sb[:], f32c[:])
                        nc.sync.dma_start(smallc_sb[:], smallc[:])
                    featst = fpool.tile([P, LC, D], F8)
                    nc.sync.dma_start(
                        featst[:],
                        feats[b, h * HALF:(h + 1) * HALF, :]
                        .rearrange("(c p) d -> p c d", p=P))
                    featsts.append(featst)
                    if u == 0:
                        nc.sync.dma_start(bfc_sb[:], bfc[:])
                for k0 in range(0, N_BF, 2):
                    nc.sync.dma_start(
                        WT_sb[:, k0:k0 + 2, :],
                        WT[k0 * P:(k0 + 2) * P, :]
                        .rearrange("(j p) n -> p j n", p=P))
                for k0 in range(0, N_E3M4, 4):
                    nc.sync.dma_start(
                        WT83_sb[:, k0:k0 + 4, :],
                        WT83[k0 * P:(k0 + 4) * P, :]
                        .rearrange("(j p) n -> p j n", p=P))
                # fc W in 3 gc-block pieces: [i,f], [g], then [o] dead
                # last -- the final-byte -> output chain is just sig_o * t2
                for g0, g1 in ((0, 8), (12, 16), (8, 12)):
                    nc.sync.dma_start(
                        WT8_sb[:, :, g0 * P:g1 * P],
                        WT8[:, g0 * P:g1 * P]
                        .rearrange("(j p) n -> p j n", p=P))

                # ============ phase A: q and beta matvecs (fp8, x1024) ======
                with tc.tile_pool(name="psA", bufs=2, space="PSUM") as psA:
                    for dc in range(DC):
                        qt = psA.tile([P, B_LOC], F32, tag="q")
                        for hc in range(HC):
                            nc.tensor.matmul(
                                qt[:],
                                f8c_sb[:, hc * D + dc * P:
                                       hc * D + (dc + 1) * P],
                                f8c_sb[:, F8C_HT + hc * B_LOC:
                                       F8C_HT + (hc + 1) * B_LOC],
                                start=(hc == 0), stop=(hc == HC - 1))
                        nc.scalar.activation(qb[:, dc, :], qt[:], AF.Identity,
                                             bias=f32c_sb[:, 16 + dc:17 + dc],
                                             scale=1.0 / S_W)
                    bps = psA.tile([1, B_LOC], F32, tag="beta", bufs=1)
                    for hc in range(HC):
                        nc.tensor.matmul(
                            bps[:],
                            f8c_sb[:, F8C_WSEL + hc:F8C_WSEL + hc + 1],
                            f8c_sb[:, F8C_HT + hc * B_LOC:
                                   F8C_HT + (hc + 1) * B_LOC],
                            start=(hc == 0), stop=(hc == HC - 1))
                    # beta = sigmoid(z) = 0.5*tanh(z/2) + 0.5 (one ACT table)
                    nc.scalar.activation(tb[:], bps[:], AF.Tanh,
                                         bias=f32c_sb[0:1, 20:21],
                                         scale=0.5 / S_W)
                    # beta' = 64*sigmoid(z): the 64 is the fp8 fc x-scale
                    nc.vector.tensor_scalar(beta_sb[:], tb[:], 32.0, 32.0,
                                            op0=ALU.mult, op1=ALU.add)

                # ================= phase B: attention + gates + LSTM ========
                with tc.tile_pool(name="pslog", bufs=2, space="PSUM") as pslog, \
                     tc.tile_pool(name="pssum", bufs=1, space="PSUM") as pssum, \
                     tc.tile_pool(name="psctx", bufs=1, space="PSUM") as psctx, \
                     tc.tile_pool(name="psg", bufs=1, space="PSUM") as psg:

                    # gate psums split per block (i|f, o, g) so the LSTM
                    # tail can read each as soon as its own writers finish;
                    # one start=True bias-broadcast preload per tile
                    g_if = psg.tile([P, 32], F32, tag="gif")
                    g_o = psg.tile([P, 16], F32, tag="go")
                    g_g = psg.tile([P, 16], F32, tag="gg")
                    for tile_, c0, c1 in ((g_if, 0, 32), (g_o, 32, 48),
                                          (g_g, 48, 64)):
                        nc.tensor.matmul(
                            tile_[:], smallc_sb[:, 0:P],
                            smallc_sb[:, SC_BRHS + c0:SC_BRHS + c1],
                            start=True, stop=False)

                    def g_dst(gc):
                        if gc < 8:
                            return g_if[:, gc * B_LOC:(gc + 1) * B_LOC]
                        if gc < 12:
                            return g_o[:, (gc - 8) * B_LOC:(gc - 7) * B_LOC]
                        return g_g[:, (gc - 12) * B_LOC:(gc - 11) * B_LOC]

                    fcA = {}
                    for b in range(B_LOC):
                        for h in range(2):
                            u = b * 2 + h
                            r = u
                            projt, featst = projts[u], featsts[u]

                            # relu(proj + q): dc0 on Pool, dc1 on DVE,
                            # dc2/dc3 on ACT (three engines in parallel)
                            hatt = hattp.tile([P, DC, HALF], BF16)
                            nc.gpsimd.tensor_scalar(
                                hatt[:, 0, :], projt[:, 0, :],
                                qb[:, 0, b:b + 1], 0.0,
                                op0=ALU.add, op1=ALU.max)
                            nc.vector.tensor_scalar(
                                hatt[:, 1, :], projt[:, 1, :],
                                qb[:, 1, b:b + 1], 0.0,
                                op0=ALU.add, op1=ALU.max)
                            for dc in (2, 3):
                                nc.scalar.activation(
                                    hatt[:, dc, :], projt[:, dc, :], AF.Relu,
                                    bias=qb[:, dc, b:b + 1])

                            # mask preload -> logitsT psum [128, 8], one start
                            lg_ps = pslog.tile([P, LC], F32)
                            nc.tensor.matmul(
                                lg_ps[:],
                                smallc_sb[0:LC, SC_MADD + r * P:
                                          SC_MADD + (r + 1) * P],
                                smallc_sb[0:LC, SC_ID8:SC_ID8 + LC],
                                start=True, stop=False)
                            # logitsT: stationary hatt chunks, moving w column
                            for dc in range(DC):
                                for lc in range(LC):
                                    nc.tensor.matmul(
                                        lg_ps[:, lc:lc + 1],
                                        hatt[:, dc, lc * P:(lc + 1) * P],
                                        bfc_sb[:, BFC_WPF + dc * 2 + h:
                                               BFC_WPF + dc * 2 + h + 1],
                                        start=False,
                                        stop=(dc == DC - 1 and lc == LC - 1))

                            # softmax: exp on [128, 8]; alpha straight to fp8
                            alpha_t = alphap.tile([P, LC, 1], F8)
                            nc.scalar.activation(alpha_t[:, :, 0], lg_ps[:],
                                                 AF.Exp)
                            sums_ps = pssum.tile([1, LC], F32, tag="sums")
                            nc.tensor.matmul(sums_ps[:], ones_f8[:],
                                             alpha_t[:, :, 0],
                                             start=True, stop=True)
                            nc.vector.tensor_reduce(
                                sums_sb[0:1, r:r + 1], sums_ps[0:1, :],
                                axis=mybir.AxisListType.X, op=ALU.add)
                            nc.vector.reciprocal(recips[0:1, r:r + 1],
                                                 sums_sb[0:1, r:r + 1])
                            nc.vector.tensor_tensor(svals[0:1, r:r + 1],
                                                    recips[0:1, r:r + 1],
                                                    beta_sb[0:1, b:b + 1],
                                                    op=ALU.mult)

                            # ctx[1, 512] += alpha_lc.T @ feats_lc (fp8)
                            ctx_ps = psctx.tile([1, D], F32)
                            for lc in range(LC):
                                nc.tensor.matmul(
                                    ctx_ps[:], alpha_t[:, lc, :],
                                    featst[:, lc, :],
                                    start=(lc == 0), stop=(lc == LC - 1))
                            if h == 0:
                                fcA_b = fcp.tile([1, D], F32, tag="fcA", bufs=2)
                                nc.vector.tensor_scalar_mul(
                                    fcA_b[:], ctx_ps[0:1, :],
                                    svals[0:1, r:r + 1])
                                fcA[b] = fcA_b
                            else:
                                fc_b = fcp.tile([1, D], F32, tag="fcB", bufs=2)
                                nc.vector.scalar_tensor_tensor(
                                    fc_b[:], ctx_ps[0:1, :], svals[0:1, r:r + 1],
                                    fcA[b][:], op0=ALU.mult, op1=ALU.add)
                                for dc in range(DC):
                                    tr_ps = pssum.tile([P, 1], F32, tag="tr",
                                                       bufs=1)
                                    nc.tensor.transpose(
                                        tr_ps[:, 0:1],
                                        fc_b[0:1, dc * P:(dc + 1) * P],
                                        ident[0:1, 0:1])
                                    nc.vector.tensor_copy(
                                        xf8_sb[:, dc * B_LOC + b:
                                               dc * B_LOC + b + 1],
                                        tr_ps[:])

                    # ========== phase C: gates + LSTM tail ==================
                    # gates matmuls in WT-arrival order; fp8 fc chunks (and
                    # the psum stop) land last, as the final W bytes arrive
                    for kc in range(N_BF):
                        for gc in range(GC):
                            nc.tensor.matmul(
                                g_dst(gc),
                                WT_sb[:, kc, gc * P:(gc + 1) * P],
                                bfc_sb[:, kc * B_LOC:(kc + 1) * B_LOC],
                                start=False, stop=False)
                    for j in range(N_E3M4):
                        kc = N_BF + j
                        for gc in range(GC):
                            nc.tensor.matmul(
                                g_dst(gc),
                                WT83_sb[:, j, gc * P:(gc + 1) * P],
                                bfc_sb[:, kc * B_LOC:(kc + 1) * B_LOC],
                                start=False, stop=False)
                    for g0, g1 in ((0, 8), (12, 16), (8, 12)):
                        for dc in range(4):
                            for gc in range(g0, g1):
                                nc.tensor.matmul(
                                    g_dst(gc),
                                    WT8_sb[:, dc, gc * P:(gc + 1) * P],
                                    xf8_sb[:, dc * B_LOC:(dc + 1) * B_LOC],
                                    start=False,
                                    stop=(dc == 3 and gc == g1 - 1))

                    # transposed LSTM tail; gate rows (permuted) = [i,f,o,g]
                    lstm = const
                    t_if = lstm.tile([P, 32], F32)
                    t_o = lstm.tile([P, 16], F32)
                    t_g = lstm.tile([P, 16], F32)
                    t2 = lstm.tile([P, 16], F32)
                    c2 = lstm.tile([P, 16], F32)
                    outt = lstm.tile([P, 32], F32)   # [c_new | h_new]

                    # sigmoid(x) = 0.5*tanh(x/2)+0.5 for i, f, o; tanh
                    # for g; psum carries S_G=256-scaled gates
                    nc.scalar.activation(t_if[:], g_if[:], AF.Tanh,
                                         scale=0.5 / S_G)
                    nc.scalar.activation(t_g[:], g_g[:], AF.Tanh,
                                         scale=1.0 / S_G)
                    nc.vector.tensor_scalar(t_if[:], t_if[:], 0.5, 0.5,
                                            op0=ALU.mult, op1=ALU.add)
                    # c_new = sig_f * c_last + sig_i * tanh_g
                    nc.vector.tensor_tensor(outt[:, 0:16], t_if[:, 16:32],
                                            f32c_sb[:, 0:16], op=ALU.mult)
                    nc.vector.tensor_tensor(c2[:], t_if[:, 0:16], t_g[:],
                                            op=ALU.mult)
                    nc.vector.tensor_tensor(outt[:, 0:16], outt[:, 0:16],
                                            c2[:], op=ALU.add)
                    nc.scalar.activation(t2[:], outt[:, 0:16], AF.Tanh)
                    # o gates land last; everything above overlapped their DMA
                    nc.scalar.activation(t_o[:], g_o[:], AF.Tanh,
                                         scale=0.5 / S_G)
                    nc.vector.tensor_scalar(t_o[:], t_o[:], 0.5, 0.5,
                                            op0=ALU.mult, op1=ALU.add)
                    nc.vector.tensor_tensor(outt[:, 16:32], t_o[:], t2[:],
                                            op=ALU.mult)
                    nc.sync.dma_start(out_d[:], outt[:])

    nc.compile()
    return nc


_NC_CACHE = None


def _get_nc():
    global _NC_CACHE
    if _NC_CACHE is None:
        _NC_CACHE = build_nc()
    return _NC_CACHE


def split_out(arr):
    """[128, 32] device layout -> (h_new [B_LOC, H], c_new [B_LOC, H])."""
    a = np.asarray(arr, np.float32).reshape(P, 2, HC, B_LOC)
    c = np.ascontiguousarray(a[:, 0].transpose(2, 1, 0).reshape(B_LOC, H))
    h = np.ascontiguousarray(a[:, 1].transpose(2, 1, 0).reshape(B_LOC, H))
    return h, c


def make_in_maps(features, features_proj, hidden_states, cell_states,
                 caption_hidden_states, w_h2a, b_h2a, w_patt, b_patt,
                 w_fatt, b_fatt, w_sel, b_sel, w_ih, w_hh, b_ih, b_hh,
                 mask, feature_idx):
    assert int(feature_idx) == FIDX
    import ml_dtypes
    f32 = np.float32
    bf16 = ml_dtypes.bfloat16
    f8 = ml_dtypes.float8_e4m3
    features = np.asarray(features, f32)
    features_proj = np.asarray(features_proj, f32)
    h_last = np.asarray(hidden_states, f32)[-1]          # [B, H]
    c_last = np.asarray(cell_states, f32)[-1]            # [B, H]
    cap = np.asarray(caption_hidden_states, f32)         # [B, H]
    mask = np.asarray(mask)

    # shared (replicated) tensors — layout-only host prep + dtype casts
    Wfull = np.concatenate([np.asarray(w_ih, f32), np.asarray(w_hh, f32)], axis=1)
    gate_perm = np.r_[0:512, 512:1024, 1536:2048, 1024:1536]   # [i, f, o, g]
    k_perm = np.r_[0:512, 1024:1536, 1536:2048, 512:1024]      # [cap,feat,h,fc]
    f83 = ml_dtypes.float8_e3m4
    b_ihh = (np.asarray(b_ih, f32) + np.asarray(b_hh, f32))[gate_perm] * S_G
    WTf = Wfull[gate_perm][:, k_perm].T                       # [k, g]
    WTh = np.ascontiguousarray(WTf[0:N_BF * P] * S_G).astype(bf16)
    WT83h = np.ascontiguousarray(
        WTf[N_BF * P:12 * P] * S_G).astype(f83)
    WT8h = np.ascontiguousarray(WTf[12 * P:] * (S_G / 64.0)).astype(f8)

    # small bf16 pack rows=16: biasT2 | brhs | ident8 | madd2 (per core)
    sc_base = np.zeros((16, 1224), f32)
    sc_base[:, 0:P] = b_ihh.reshape(GC, P)
    sc_base[:, SC_BRHS:SC_BRHS + GC * B_LOC] = np.kron(
        np.eye(GC, dtype=f32), np.ones((1, B_LOC), f32))
    sc_base[0:LC, SC_ID8:SC_ID8 + LC] = np.eye(LC, dtype=f32)

    # f8 pack: w_h2a (hc-major) | w_sel | hT8 (hT8 filled per core)
    f8c_w = np.zeros((P, 2068), f32)
    w_h2aT = np.asarray(w_h2a, f32).T * S_W                    # [H, D]
    f8c_w[:, 0:2048] = w_h2aT.reshape(HC, P, D).transpose(1, 0, 2) \
        .reshape(P, 2048)
    f8c_w[:, F8C_WSEL:F8C_WSEL + HC] = (
        np.asarray(w_sel, f32).T * S_W).reshape(HC, P).T

    w_pf = np.stack([np.asarray(w_patt, f32)[0], np.asarray(w_fatt, f32)[0]],
                    axis=1)                                    # [D, 2]
    madd = np.where(mask, f32(0), f32(-1e30))                  # [B, L]

    in_maps = []
    for c in range(N_CORES):
        sl = slice(c * B_LOC, (c + 1) * B_LOC)
        sc = sc_base.copy()
        # madd2[lc, (b*2+h)*128 + p] = madd_loc[b, h*HALF + lc*128 + p]
        m = madd[sl].reshape(B_LOC, 2, LC, P)
        sc[0:LC, SC_MADD:SC_MADD + 2 * B_LOC * P] = (
            m.transpose(2, 0, 1, 3).reshape(LC, 2 * B_LOC * P))

        f8cc = f8c_w.copy()
        f8cc[:, F8C_HT:F8C_HT + 16] = (
            h_last[sl].T.reshape(HC, P, B_LOC).transpose(1, 0, 2)
            .reshape(P, 16))

        bfc = np.zeros((P, 72), f32)
        xh = np.stack([cap[sl], features[sl, FIDX, :], h_last[sl]], axis=0)
        # xh [3, B_LOC, H] -> bfc[p, kc*4+b] for kc blocks cap/feat/h
        bfc[:, 0:48] = xh.reshape(3, B_LOC, HC, P).transpose(3, 0, 2, 1) \
            .reshape(P, 48)
        bfc[:, BFC_WPF:BFC_WPF + 8] = w_pf.reshape(DC, P, 2) \
            .transpose(1, 0, 2).reshape(P, 8)

        f32cc = np.zeros((P, 21), f32)
        f32cc[:, 0:16] = c_last[sl].T.reshape(HC, P, B_LOC) \
            .transpose(1, 0, 2).reshape(P, 16)
        f32cc[:, 16:20] = np.asarray(b_h2a, f32).reshape(DC, P).T
        f32cc[0, 20] = np.asarray(b_sel, f32).reshape(-1)[0] * 0.5

        in_maps.append({
            "projT": np.ascontiguousarray(
                features_proj[sl].transpose(0, 2, 1)).astype(f8),
            "feats": np.ascontiguousarray(features[sl]).astype(f8),
            "WT": WTh,
            "WT83": WT83h,
            "WT8": WT8h,
            "f8c": f8cc.astype(f8),
            "f32c": f32cc,
            "bfc": bfc.astype(bf16),
            "smallc": sc.astype(bf16),
        })
    return in_maps


def run(trace=False, **inputs):
    nc = _get_nc()
    in_maps = make_in_maps(**inputs)
    res = run_bass_kernel_spmd(nc, in_maps, core_ids=list(range(N_CORES)),
                               trace=trace)
    hs, cs = [], []
    for c in range(N_CORES):
        h, cc = split_out(res.results[c]["out"])
        hs.append(h)
        cs.append(cc)
    return (np.concatenate(hs)[None], np.concatenate(cs)[None]), res


def kernel(**inputs):
    out, _ = run(trace=False, **inputs)
    return out
